# revision 1
# baseline (speedup 1.0000x reference)
"""EdgeConv (PyG, aggr='max') Trainium2 kernel, 8-core SPMD.

Math: out_i = max_{e: dst(e)=i} relu(x_i @ W1.T + (x_src(e) - x_i) @ W2.T + b)
with W = [W1 | W2].  Rewriting:
    msg_e = relu(A_i + g_src(e)),  A = x @ (W1-W2).T + b,  g = x @ W2.T
Since A_i is constant within segment i and relu is monotone:
    out_i = relu(A_i + max_e g_src(e))
The reference's dst is repeat(arange(N), DEG) (fixed-degree kNN-style graph),
so segments are 16 consecutive edges; segment-max becomes a grouped reduce.

Two SPMD launches on 8 cores:
  L1 (node-parallel): per-core 6250-node shard computes hT = wcat.T @ xT in
     channel-major orientation. The host pre-transposes x (bf16), so L1 is
     13 wide PE matmuls (512-column PSUM-bank tiles, no on-device
     transposes); A-channels get the bias via DVE, g-channels are copied to
     bf16 by ACT. Outputs are written channel-major contiguous.
  L2 (edge-parallel): per-core 100k-edge shard bulk-gathers 256B bf16
     row-PAIRS [g_{2r} | g_{2r+1}] by src>>1 with non-transpose dma_gather
     (one descriptor per edge; src>>1 <= 24999 fits int16; pad positions
     read the sentinel pair-row NPAIR = -3e38; the all-padding final
     node-tile is dropped, so the last chunk is half-size). The parity
     half-select overwrites the even half in place via copy_predicated with
     a host-precomputed uint8 mask. A host-side edge permutation lands node
     n's 16 slots at partition n%128, slots 16*(n//128)+k, so the segment
     max is a bf16 max-tree (packed 2x DVE mode). Results accumulate in
     SBUF (bf16) and are written in overlapped pieces; the host converts
     to f32.
"""

import numpy as np
import ml_dtypes

BF16 = ml_dtypes.bfloat16

N_NODES = 50000
DEG = 16
C = 64
N_CORES = 8
NSH = N_NODES // N_CORES  # 6250 nodes per core
P = 128
TCH = 2  # node-tiles per L2 chunk
CHUNK = TCH * P  # 256 nodes per L2 chunk
NSH_PAD = 6400  # 25 chunks * 256; 50 tiles * 128
NT = NSH_PAD // P  # 50
NCHUNKS = NSH_PAD // CHUNK  # 25
NI = CHUNK * DEG  # 4096 gather positions per chunk
NPAIR = N_NODES // 2  # 256B row-pairs in the gather table
SENT = -3.0e38
SUP = 512  # dense supertile columns (one PSUM bank)

_cache = {}


def _build_dense():
    import concourse.bacc as bacc
    import concourse.mybir as mybir
    from concourse.tile import TileContext

    nc = bacc.Bacc("TRN2", target_bir_lowering=False, debug=False)
    f32 = mybir.dt.float32
    bf16 = mybir.dt.bfloat16
    # xt: column n = x[shard_base+n], plus a trailing ones row (bias input)
    xt = nc.dram_tensor("xt", [C + 1, NSH_PAD], bf16, kind="ExternalInput")
    # wcat[in, 0:64] = (W1-W2).T ; wcat[in, 64:128] = W2.T ; row C = [b | 0]
    wcat = nc.dram_tensor("wcat", [C + 1, 2 * C], bf16, kind="ExternalInput")
    # gat[0:64] = A channels (bias included), gat[64:128] = g channels
    gat = nc.dram_tensor("gat", [2 * C, NSH_PAD], bf16, kind="ExternalOutput")

    nsup = NSH_PAD // SUP  # 12 supertiles of 512 + 1 of 256
    rem = NSH_PAD - nsup * SUP
    spans = [(i * SUP, SUP) for i in range(nsup)] + ([(nsup * SUP, rem)] if rem else [])

    with TileContext(nc) as tc:
        with (
            tc.tile_pool(name="const", bufs=1) as cpool,
            tc.tile_pool(name="sbuf", bufs=1) as pool,
            tc.tile_pool(name="psum", bufs=6, space="PSUM") as psum,
        ):
            w_sb = cpool.tile([C + 1, 2 * C], bf16)
            nc.sync.dma_start(out=w_sb[:], in_=wcat[:])
            # preload the ACT function table while xt streams in
            warm = cpool.tile([1, 2], f32)
            nc.vector.memset(warm[:], 0.0)
            warm2 = cpool.tile([1, 2], f32)
            nc.scalar.copy(out=warm2[:], in_=warm[:])
            xt_sb = pool.tile([C + 1, NSH_PAD], bf16, tag="xt")
            xsp = [(0, SUP), (SUP, 4 * SUP), (4 * SUP, 8 * SUP), (8 * SUP, NSH_PAD)]
            for k, (a, b) in enumerate(xsp):
                nc.sync.dma_start(out=xt_sb[:, a:b], in_=xt[:, a:b])
            # staged output tiles: each piece is written to HBM as soon as its
            # supertiles are done, overlapping the remaining compute
            pieces = [(0, 5), (5, 10), (10, 12), (12, len(spans))]
            ga_t = []
            for q0, q1 in pieces:
                w = spans[q1 - 1][0] + spans[q1 - 1][1] - spans[q0][0]
                ga_piece = pool.tile([2 * C, w], bf16, tag=f"ga{q0}", name=f"ga{q0}")
                ga_t.append(ga_piece)
            for i, (s0, sl) in enumerate(spans):
                cols = slice(s0, s0 + sl)
                ps = psum.tile([2 * C, SUP], f32, tag="h")
                nc.tensor.matmul(
                    out=ps[:, 0:sl], lhsT=w_sb[:], rhs=xt_sb[:, cols],
                    start=True, stop=True,
                )
                pi = next(j for j, (q0, q1) in enumerate(pieces) if q0 <= i < q1)
                base = spans[pieces[pi][0]][0]
                dst = ga_t[pi][:, s0 - base : s0 - base + sl]
                if i % 2 == 0:
                    nc.scalar.copy(out=dst, in_=ps[:, 0:sl])
                else:
                    nc.vector.tensor_copy(out=dst, in_=ps[:, 0:sl])
                if i == pieces[pi][1] - 1:
                    hi = spans[pieces[pi][1] - 1][0] + spans[pieces[pi][1] - 1][1]
                    nc.sync.dma_start(out=gat[:, base:hi], in_=ga_t[pi][:])
    nc.compile()
    return nc


def _build_gather():
    import concourse.bacc as bacc
    import concourse.mybir as mybir
    from concourse.tile import TileContext

    nc = bacc.Bacc("TRN2", target_bir_lowering=False, debug=False)
    f32 = mybir.dt.float32
    bf16 = mybir.dt.bfloat16
    i16 = mybir.dt.int16
    u8 = mybir.dt.uint8
    mx = mybir.AluOpType.max
    # pair table: row r = [g_{2r} | g_{2r+1}] (256B); row NPAIR = sentinel
    gpair = nc.dram_tensor("gpair", [NPAIR + 1, 2 * C], bf16, kind="ExternalInput")
    idx = nc.dram_tensor("idx", [P, NCHUNKS * (NI // 16)], i16, kind="ExternalInput")
    msk = nc.dram_tensor("msk", [P, NCHUNKS * TCH * DEG], u8, kind="ExternalInput")
    ash = nc.dram_tensor("ash", [P, NT * C], bf16, kind="ExternalInput")
    osh = nc.dram_tensor("osh", [P, (NT - 1) * C], bf16, kind="ExternalOutput")

    with TileContext(nc) as tc:
        with (
            tc.tile_pool(name="sbuf", bufs=1) as pool,
            tc.tile_pool(name="gat", bufs=4) as gpool,
        ):
            # idx split so chunk 0 can start gathering immediately; msk and
            # the second ash half slot into the DMA queue before/after the
            # long idx tail without delaying the first transfers
            S = NI // 16
    
            idxA = pool.tile([P, 6, S], i16, tag="idxA")
            nc.sync.dma_start(
                out=idxA[:],
                in_=idx[:, 0 : 6 * S].rearrange("p (h s) -> p h s", h=6),
            )
            msk_all = pool.tile([P, NCHUNKS, TCH * DEG], u8, tag="msk")
            nc.sync.dma_start(
                out=msk_all[:], in_=msk[:].rearrange("p (h s) -> p h s", h=NCHUNKS)
            )
            a_all = pool.tile([P, NT, C], bf16, tag="a")
            nc.sync.dma_start(
                out=a_all[:, 0 : NT // 2, :],
                in_=ash[:, 0 : NT // 2 * C].rearrange("p (t c) -> p t c", t=NT // 2),
            )
            idxB = pool.tile([P, NCHUNKS - 6, S], i16, tag="idxB")
            o_all = pool.tile([P, NT - 1, C], bf16, tag="o")
            # last chunk is half-size: node-tile 49 (6272-6399) is all padding
            for ch in range(NCHUNKS):
                tch = TCH if ch < NCHUNKS - 1 else 1
                ni = tch * P * DEG
                # position j lands at partition j%128, slot j//128; each slot
                # holds a 256B row-pair [even | odd]
                gath = gpool.tile([P, TCH, DEG, 2, C], bf16, tag="gath")
                nc.gpsimd.dma_gather(
                    out_ap=gath[:, 0:tch, :, :, :].rearrange(
                        "p t k two c -> p (t k) (two c)"
                    ),
                    in_ap=gpair[:],
                    idxs_ap=(
                        idxA[:, ch, 0 : ni // 16]
                        if ch < 6
                        else idxB[:, ch - 6, 0 : ni // 16]
                    ),
                    num_idxs=ni,
                    num_idxs_reg=ni,
                    elem_size=2 * C,
                    transpose=False,
                    queue_num=0,
                    single_packet=False,
                )
                if ch == 2:
                    nc.sync.dma_start(
                        out=idxB[:],
                        in_=idx[:, 6 * S :].rearrange(
                            "p (h s) -> p h s", h=NCHUNKS - 6
                        ),
                    )
                    nc.sync.dma_start(
                        out=a_all[:, NT // 2 :, :],
                        in_=ash[:, NT // 2 * C :].rearrange(
                            "p (t c) -> p t c", t=NT - NT // 2
                        ),
                    )
                # parity select in place: odd overwrites even where mask=1
                nc.vector.copy_predicated(
                    out=gath[:, 0:tch, :, 0, :],
                    mask=msk_all[:, ch, 0 : tch * DEG]
                    .rearrange("p (t k) -> p t k", k=DEG)
                    .to_broadcast([P, tch, DEG, C]),
                    data=gath[:, 0:tch, :, 1, :],
                )
                # segment max as a bf16 max-tree (packed free dim -> 2x DVE)
                t1 = gpool.tile([P, TCH, 8, C], bf16, tag="t1")
                nc.vector.tensor_tensor(
                    out=t1[:, 0:tch],
                    in0=gath[:, 0:tch, 0:8, 0, :],
                    in1=gath[:, 0:tch, 8:16, 0, :],
                    op=mx,
                )
                t2 = gpool.tile([P, TCH, 4, C], bf16, tag="t2")
                nc.vector.tensor_tensor(
                    out=t2[:, 0:tch],
                    in0=t1[:, 0:tch, 0:4, :],
                    in1=t1[:, 0:tch, 4:8, :],
                    op=mx,
                )
                t3 = gpool.tile([P, TCH, 2, C], bf16, tag="t3")
                nc.vector.tensor_tensor(
                    out=t3[:, 0:tch],
                    in0=t2[:, 0:tch, 0:2, :],
                    in1=t2[:, 0:tch, 2:4, :],
                    op=mx,
                )
                m_sb = gpool.tile([P, TCH, C], f32, tag="m")
                nc.vector.tensor_tensor(
                    out=m_sb[:, 0:tch],
                    in0=t3[:, 0:tch, 0, :],
                    in1=t3[:, 0:tch, 1, :],
                    op=mx,
                )
                s_sb = gpool.tile([P, TCH, C], f32, tag="s")
                nc.vector.tensor_add(
                    out=s_sb[:, 0:tch],
                    in0=m_sb[:, 0:tch],
                    in1=a_all[:, ch * TCH : ch * TCH + tch, :],
                )
                nc.scalar.activation(
                    out=o_all[:, ch * TCH : ch * TCH + tch, :],
                    in_=s_sb[:, 0:tch],
                    func=mybir.ActivationFunctionType.Relu,
                )
                if ch in (11, 19, 23):
                    lo = {11: 0, 19: 24, 23: 40}[ch]
                    hi = (ch + 1) * TCH
                    nc.sync.dma_start(
                        out=osh[:, lo * C : hi * C].rearrange(
                            "p (t c) -> p t c", t=hi - lo
                        ),
                        in_=o_all[:, lo:hi, :],
                    )
            nc.sync.dma_start(
                out=osh[:, 48 * C : (NT - 1) * C].rearrange(
                    "p (t c) -> p t c", t=NT - 1 - 48
                ),
                in_=o_all[:, 48:, :],
            )
    nc.compile()
    return nc


def _make_indices(src_pad):
    """src_pad: [NSH_PAD, DEG] int64 node ids (pad rows = -1).
    Returns (idx, msk): pair-row indices (src>>1, sentinel NPAIR for pads) in
    dma_gather's index layout (16 partitions), and the odd-parity mask in dest
    layout [128, slots]. Position j of chunk ch covers node n_c = j%128 +
    128*(j//128 // DEG) ... specifically j = (DEG*(n_c//128)+k)*128 + (n_c%128)."""
    s = src_pad.reshape(NCHUNKS, TCH, P, DEG)
    flat = np.transpose(s, (0, 1, 3, 2)).reshape(NCHUNKS, NI)  # [ch, (t k p)]
    pidx = np.where(flat >= 0, flat >> 1, NPAIR).astype(np.int16)
    par = np.where(flat >= 0, flat & 1, 0).astype(np.uint8)
    # index layout: position j -> [j%16, j//16], replicated 8x down partitions
    a = np.swapaxes(pidx.reshape(NCHUNKS, NI // 16, 16), 1, 2)  # [ch, 16, s]
    idx = np.ascontiguousarray(
        np.tile(a, (1, 8, 1)).transpose(1, 0, 2).reshape(P, NCHUNKS * (NI // 16))
    )
    # mask layout: dest [partition j%128, slot j//128]
    m = np.swapaxes(par.reshape(NCHUNKS, TCH * DEG, P), 1, 2)
    msk = np.ascontiguousarray(m.transpose(1, 0, 2).reshape(P, NCHUNKS * TCH * DEG))
    return idx, msk


def _numpy_fallback(x, edge_index, W, b):
    src, dst = edge_index[0], edge_index[1]
    V1 = W[:, :C] - W[:, C:]
    V2 = W[:, C:]
    A = x @ V1.T + b
    g = x @ V2.T
    out = np.full((x.shape[0], C), -np.inf, dtype=np.float32)
    msg = np.maximum(A[dst] + g[src], 0.0)
    np.maximum.at(out, dst, msg)
    return np.where(np.isneginf(out), 0.0, out).astype(np.float32)


def _run_spmd(nc, in_maps):
    # the shared axon device occasionally reports a transient
    # NRT_EXEC_UNIT_UNRECOVERABLE on a cold first launch; retry once
    import time
    from concourse.bass_utils import run_bass_kernel_spmd

    try:
        return run_bass_kernel_spmd(nc, in_maps, core_ids=list(range(N_CORES)))
    except Exception:
        time.sleep(10.0)
        return run_bass_kernel_spmd(nc, in_maps, core_ids=list(range(N_CORES)))


def kernel(x, edge_index, edge_attr, W, b):

    x = np.ascontiguousarray(x, dtype=np.float32)
    edge_index = np.ascontiguousarray(edge_index, dtype=np.int32)
    W = np.ascontiguousarray(W, dtype=np.float32)
    b = np.ascontiguousarray(b, dtype=np.float32)

    expected_dst = np.repeat(np.arange(N_NODES, dtype=np.int32), DEG)
    if (
        x.shape != (N_NODES, C)
        or edge_index.shape != (2, N_NODES * DEG)
        or not np.array_equal(edge_index[1], expected_dst)
    ):
        return _numpy_fallback(x, edge_index, W, b)

    if "dense" not in _cache:
        _cache["dense"] = _build_dense()
    if "gather" not in _cache:
        _cache["gather"] = _build_gather()

    # ---- Launch 1: node-parallel dense phase (channel-major) ----
    # wcat[in, 0:64] = (W1-W2).T, wcat[in, 64:128] = W2.T, row C = [b | 0]
    W1, W2 = W[:, :C], W[:, C:]
    wcat = np.concatenate(
        [
            np.concatenate([(W1 - W2).T, W2.T], axis=1),
            np.concatenate([b, np.zeros(C, np.float32)]).reshape(1, 2 * C),
        ],
        axis=0,
    ).astype(BF16)
    xtp = np.zeros((C + 1, N_CORES, NSH_PAD), dtype=BF16)
    xtp[:C, :, :NSH] = (
        x.astype(BF16).reshape(N_CORES, NSH, C).transpose(2, 0, 1)
    )
    xtp[C] = 1.0
    in1 = [
        {"xt": np.ascontiguousarray(xtp[:, c, :]), "wcat": wcat}
        for c in range(N_CORES)
    ]
    r1 = _run_spmd(_cache["dense"], in1)

    # host: assemble the full pair table [g_2r | g_2r+1] + sentinel row
    g_full = np.concatenate(
        [r1.results[c]["gat"][C:, :NSH].T for c in range(N_CORES)], axis=0
    )  # [N_NODES, C] bf16
    gpair = np.concatenate(
        [
            np.ascontiguousarray(g_full).reshape(NPAIR, 2 * C),
            np.full((1, 2 * C), SENT, dtype=BF16),
        ],
        axis=0,
    )
    gpair = np.ascontiguousarray(gpair)

    # ---- Launch 2: edge-parallel gather + segment max ----
    src = edge_index[0]
    in2 = []
    for c in range(N_CORES):
        s = np.full((NSH_PAD, DEG), -1, dtype=np.int64)
        s[:NSH] = src[c * NSH * DEG : (c + 1) * NSH * DEG].reshape(NSH, DEG)
        idx, msk = _make_indices(s)
        # ash[p, t*C+c] = A[128*t + p, c] of this core's shard
        at = r1.results[c]["gat"][:C]  # [C, NSH_PAD] bf16
        ash = np.ascontiguousarray(
            at.T.reshape(NT, P, C).transpose(1, 0, 2).reshape(P, NT * C)
        )
        in2.append({"gpair": gpair, "idx": idx, "msk": msk, "ash": ash})
    r2 = _run_spmd(_cache["gather"], in2)

    out = np.empty((N_CORES, NSH, C), dtype=np.float32)
    for c in range(N_CORES):
        o = (
            r2.results[c]["osh"]
            .reshape(P, NT - 1, C)
            .transpose(1, 0, 2)
            .reshape((NT - 1) * P, C)
        )
        out[c] = o[:NSH].astype(np.float32)
    _cache["last_results"] = (r1, r2)
    return out.reshape(N_NODES, C)



# revision 4
# speedup vs baseline: 1.9518x; 1.9518x over previous
"""EdgeConv (PyG, aggr='max') Trainium2 kernel, 8-core SPMD.

Math: out_i = max_{e: dst(e)=i} relu(x_i @ W1.T + (x_src(e) - x_i) @ W2.T + b)
with W = [W1 | W2].  Rewriting:
    msg_e = A_i + g_src(e),  A = x @ (W1-W2).T + b,  g = x @ W2.T
Since A_i is constant within segment i and relu is monotone:
    out_i = relu(A_i + max_e g_src(e))

Sharding: edges are partitioned across cores BY SOURCE RANGE (core c owns
srcs [6250c, 6250(c+1))), so each core's entire gather table is its own
locally-computed g-shard kept in SBUF -- no HBM gather at all.  The table is
channel-paired int32 [128, 6400]: partition p holds channels (p%32, p%32+32)
packed as 2xbf16, replicated over 4 independent 32-partition "streams".  A
single GPSIMD ap_gather column then fetches a full 64-channel row for 4
different edges at once (one per stream) at ~0.35 ns/edge -- 4x cheaper than
the DMA-descriptor path and on an otherwise idle engine.

Each core's destination nodes are grouped by their per-core edge count k
(host-side reorder); the segment max becomes regular k-window max trees on
DVE over the gathered columns.  Per-core partial maxes [128, R] are written
out; the host un-permutes and np.maximum-merges them across cores (pure
unshard glue).  A second small node-sharded launch computes
relu(A + merged_max) and writes the final bf16 output.

Launch 1 (gather): xt/wlo/whi -> PE builds the paired table; 4 ap_gather
chunks of ~6400 columns pipeline with DVE max-tree spans; osh partials.
Launch 2 (dense): 13 supertile matmuls for A, DVE adds the merged max,
ACT applies relu.
"""

import math

import numpy as np
import ml_dtypes

BF16 = ml_dtypes.bfloat16

N_NODES = 50000
DEG = 16
C = 64
N_CORES = 8
NSH = N_NODES // N_CORES  # 6250 src nodes per core
NLOC = 6400  # padded local node count (table columns / L2 shard columns)
NSTR = 4  # gather streams (32 partitions each)
KMAX = DEG
SUP = 512  # supertile columns (one PSUM bank)
KC_TARGET = NLOC  # gather chunk columns ~ table size (cost floor)

_cache = {}


# ---------------------------------------------------------------------------
# host-side layout
# ---------------------------------------------------------------------------

def _host_prep(src, dst):
    """Compute the SPMD-uniform column layout from the actual edge list.

    Returns cfg (hashable, shapes for the kernel builder) and per-core host
    data (idx arrays, decode maps).
    """
    E = src.shape[0]
    core = (src // NSH).astype(np.int64)
    n64 = dst.astype(np.int64)
    kmat = np.bincount(n64 * N_CORES + core, minlength=N_NODES * N_CORES)
    kmat = kmat.reshape(N_NODES, N_CORES)  # [N, 8] per-(node, core) edge count

    # group nodes by k per core; round-robin nodes over 4 streams
    s_nc = np.zeros((N_CORES, N_NODES), np.int64)
    q_nc = np.zeros((N_CORES, N_NODES), np.int64)
    caps = np.zeros(KMAX + 1, np.int64)  # per-(k, stream) node capacity
    core_nodes = []  # per core: (nodes, k of nodes)
    for c in range(N_CORES):
        kc = kmat[:, c]
        nz = np.nonzero(kc)[0]
        core_nodes.append((nz, kc[nz]))
        for k in range(1, KMAX + 1):
            nodes = nz[kc[nz] == k]
            m = len(nodes)
            if m == 0:
                continue
            r = np.arange(m)
            s_nc[c, nodes] = r % NSTR
            q_nc[c, nodes] = r // NSTR
            caps[k] = max(caps[k], -(-m // NSTR))
    # pad caps so every k-group is 128-column aligned (the GPSIMD gather
    # wants 4B-aligned index slices and block-aligned work splits)
    for k in range(1, KMAX + 1):
        if caps[k]:
            step = 128 // math.gcd(k, 128)
            caps[k] = -(-caps[k] // step) * step
    offk = np.zeros(KMAX + 2, np.int64)
    rankb = np.zeros(KMAX + 2, np.int64)
    for k in range(1, KMAX + 1):
        offk[k + 1] = offk[k] + caps[k] * k
        rankb[k + 1] = rankb[k] + caps[k]
    ctot = int(offk[KMAX + 1])
    rtot = int(rankb[KMAX + 1])

    # node-start columns in rank order (for chunk boundary search)
    starts = np.concatenate(
        [offk[k] + np.arange(caps[k]) * k for k in range(1, KMAX + 1) if caps[k]]
    )
    # chunk boundaries: near equal quarters, node-aligned and 16-aligned
    nch = max(1, int(round(ctot / KC_TARGET)))
    aligned = starts[starts % 128 == 0]
    bounds = [0]
    for i in range(1, nch):
        t = ctot * i // nch
        j = int(np.argmin(np.abs(aligned - t)))
        b = int(aligned[j])
        if b <= bounds[-1]:
            continue
        bounds.append(b)
    bounds.append(ctot)
    # chunk rank bounds
    rb = [int(np.searchsorted(starts, b)) for b in bounds]
    chunks = []
    for i in range(len(bounds) - 1):
        chunks.append((bounds[i], bounds[i + 1] - bounds[i], rb[i], rb[i + 1]))

    # reduce spans: (chunk_idx, col0, n_nodes, k, rank0), split at chunk bounds
    spans = []
    for k in range(1, KMAX + 1):
        if not caps[k]:
            continue
        g0, g1 = int(offk[k]), int(offk[k + 1])
        for ci, (b0, kc_, _, _) in enumerate(chunks):
            lo, hi = max(g0, b0), min(g1, b0 + kc_)
            if lo >= hi:
                continue
            spans.append(
                (ci, lo, (hi - lo) // k, k, int(rankb[k] + (lo - g0) // k))
            )

    # per-edge column assignment
    key = core * N_NODES + n64
    order = np.argsort(key, kind="stable")
    sk = key[order]
    first_new = np.r_[True, sk[1:] != sk[:-1]]
    run_id = np.cumsum(first_new) - 1
    run_start = np.nonzero(first_new)[0]
    j_sorted = np.arange(E) - run_start[run_id]
    occ = np.empty(E, np.int64)
    occ[order] = j_sorted
    k_e = kmat[n64, core]
    col_e = offk[k_e] + q_nc[core, n64] * k_e + occ
    s_e = s_nc[core, n64]
    ls_e = (src - core * NSH).astype(np.int64)

    idxs = np.zeros((N_CORES, NSTR, ctot), np.int16)
    idxs[core, s_e, col_e] = ls_e

    # wrapped idx layout [128, ctot//16]: group g (partitions 16g..16g+16)
    # carries stream g//2's list, element j at [16g + j%16, j//16]
    idx_wrapped = np.empty((N_CORES, 128, ctot // 16), np.int16)
    for c in range(N_CORES):
        a = idxs[c].reshape(NSTR, ctot // 16, 16)
        for g in range(8):
            idx_wrapped[c, 16 * g : 16 * (g + 1), :] = a[g // 2].T

    cfg = (ctot, rtot, tuple(chunks), tuple(spans))
    host = {
        "idx": idx_wrapped,
        "core_nodes": core_nodes,
        "s_nc": s_nc,
        "q_nc": q_nc,
        "rankb": rankb,
        "rtot": rtot,
    }
    return cfg, host


# ---------------------------------------------------------------------------
# launch 1: src-sharded gather + per-core segment max partials
# ---------------------------------------------------------------------------

def _build_gather(cfg):
    import concourse.bacc as bacc
    import concourse.mybir as mybir
    from concourse.tile import TileContext

    ctot, rtot, chunks, spans = cfg
    kc_max = max(kc for _, kc, _, _ in chunks)

    nc = bacc.Bacc("TRN2", target_bir_lowering=False, debug=False)
    f32 = mybir.dt.float32
    bf16 = mybir.dt.bfloat16
    i32 = mybir.dt.int32
    i16 = mybir.dt.int16
    mx = mybir.AluOpType.max

    xt = nc.dram_tensor("xt", [C, NLOC], bf16, kind="ExternalInput")
    wlo = nc.dram_tensor("wlo", [C, 128], bf16, kind="ExternalInput")
    whi = nc.dram_tensor("whi", [C, 128], bf16, kind="ExternalInput")
    idx = nc.dram_tensor("idx", [128, ctot // 16], i16, kind="ExternalInput")
    osh = nc.dram_tensor("osh", [128, 2 * rtot], bf16, kind="ExternalOutput")

    nsup = NLOC // SUP  # 12.5 -> handle tail below
    sup_spans = [(i * SUP, SUP) for i in range(NLOC // SUP)]
    if NLOC % SUP:
        sup_spans.append((NLOC // SUP * SUP, NLOC % SUP))

    with TileContext(nc) as tc:
        with (
            tc.tile_pool(name="const", bufs=1) as cpool,
            tc.tile_pool(name="sbuf", bufs=1) as pool,
            tc.tile_pool(name="gat", bufs=3) as gpool,
            tc.tile_pool(name="psum", bufs=4, space="PSUM") as psum,
        ):
            # small loads first: weights + the gather index list
            wlo_sb = cpool.tile([C, 128], bf16)
            nc.sync.dma_start(out=wlo_sb[:], in_=wlo[:])
            whi_sb = cpool.tile([C, 128], bf16)
            nc.sync.dma_start(out=whi_sb[:], in_=whi[:])
            idx_sb = pool.tile([128, ctot // 16], i16, tag="idx")
            nc.sync.dma_start(out=idx_sb[:], in_=idx[:])
            xt_sb = pool.tile([C, NLOC], bf16, tag="xt")
            for a, b in ((0, 2 * SUP), (2 * SUP, 8 * SUP), (8 * SUP, NLOC)):
                nc.sync.dma_start(out=xt_sb[:, a:b], in_=xt[:, a:b])

            # paired g table: int32[p, n] = (g[n, p%32] , g[n, p%32+32])
            tbl = pool.tile([128, NLOC], i32, tag="tbl")
            tbl_bf = tbl[:].bitcast(bf16).rearrange("p (n t) -> p n t", t=2)
            for ti, (s0, sl) in enumerate(sup_spans):
                cols = slice(s0, s0 + sl)
                ps_lo = psum.tile([128, SUP], f32, tag="plo")
                nc.tensor.matmul(
                    out=ps_lo[:, 0:sl], lhsT=wlo_sb[:], rhs=xt_sb[:, cols],
                    start=True, stop=True,
                )
                nc.scalar.copy(out=tbl_bf[:, cols, 0], in_=ps_lo[:, 0:sl])
                ps_hi = psum.tile([128, SUP], f32, tag="phi")
                nc.tensor.matmul(
                    out=ps_hi[:, 0:sl], lhsT=whi_sb[:], rhs=xt_sb[:, cols],
                    start=True, stop=True,
                )
                nc.vector.tensor_copy(out=tbl_bf[:, cols, 1], in_=ps_hi[:, 0:sl])

            osh_buf = pool.tile([128, rtot], i32, tag="oshb")
            osh_bf = osh_buf[:].bitcast(bf16).rearrange("p (n t) -> p n t", t=2)

            for ci, (b0, kc, cr0, cr1) in enumerate(chunks):
                g = gpool.tile([128, kc_max], i32, tag="g")
                nc.gpsimd.ap_gather(
                    out_ap=g[:, 0:kc].rearrange("p (n d) -> p n d", d=1),
                    in_ap=tbl[:].rearrange("p (n d) -> p n d", d=1),
                    idxs_ap=idx_sb[:, b0 // 16 : (b0 + kc) // 16],
                    channels=128, num_elems=NLOC, d=1, num_idxs=kc,
                )
                g_bf = g[:].bitcast(bf16)
                for sci, col0, nn, k, r0 in spans:
                    if sci != ci:
                        continue
                    l0 = col0 - b0
                    v = g_bf[:, 2 * l0 : 2 * (l0 + nn * k)].rearrange(
                        "p (n k t) -> p n k t", k=k, t=2
                    )
                    dst = osh_bf[:, r0 : r0 + nn, :]
                    if k == 1:
                        nc.vector.tensor_copy(out=dst, in_=v[:, :, 0, :])
                        continue
                    j = k
                    while j > 2:
                        if j % 2:
                            nc.vector.tensor_tensor(
                                out=v[:, :, 0, :], in0=v[:, :, 0, :],
                                in1=v[:, :, j - 1, :], op=mx,
                            )
                            j -= 1
                        m = j // 2
                        if j > 2:
                            nc.vector.tensor_tensor(
                                out=v[:, :, 0:m, :], in0=v[:, :, 0:m, :],
                                in1=v[:, :, m : 2 * m, :], op=mx,
                            )
                            j = m
                    nc.vector.tensor_tensor(
                        out=dst, in0=v[:, :, 0, :], in1=v[:, :, 1, :], op=mx
                    )
                if cr1 > cr0:
                    nc.sync.dma_start(
                        out=osh[:, 2 * cr0 : 2 * cr1],
                        in_=osh_buf[:].bitcast(bf16)[:, 2 * cr0 : 2 * cr1],
                    )
    nc.compile()
    return nc


# ---------------------------------------------------------------------------
# launch 2: node-sharded A + merged max, relu
# ---------------------------------------------------------------------------

def _build_dense():
    import concourse.bacc as bacc
    import concourse.mybir as mybir
    from concourse.tile import TileContext

    nc = bacc.Bacc("TRN2", target_bir_lowering=False, debug=False)
    f32 = mybir.dt.float32
    bf16 = mybir.dt.bfloat16

    xt = nc.dram_tensor("xt", [C + 1, NLOC], bf16, kind="ExternalInput")
    wa = nc.dram_tensor("wa", [C + 1, C], bf16, kind="ExternalInput")
    mg = nc.dram_tensor("mg", [C, NLOC], bf16, kind="ExternalInput")
    osh = nc.dram_tensor("osh", [C, NLOC], bf16, kind="ExternalOutput")

    sup_spans = [(i * SUP, SUP) for i in range(NLOC // SUP)]
    if NLOC % SUP:
        sup_spans.append((NLOC // SUP * SUP, NLOC % SUP))

    with TileContext(nc) as tc:
        with (
            tc.tile_pool(name="const", bufs=1) as cpool,
            tc.tile_pool(name="sbuf", bufs=1) as pool,
            tc.tile_pool(name="psum", bufs=4, space="PSUM") as psum,
        ):
            wa_sb = cpool.tile([C + 1, C], bf16)
            nc.sync.dma_start(out=wa_sb[:], in_=wa[:])
            # preload the ACT function table while inputs stream in
            warm = cpool.tile([1, 2], f32)
            nc.vector.memset(warm[:], 0.0)
            warm2 = cpool.tile([1, 2], f32)
            nc.scalar.activation(
                out=warm2[:], in_=warm[:],
                func=mybir.ActivationFunctionType.Relu,
            )
            xt_sb = pool.tile([C + 1, NLOC], bf16, tag="xt")
            mg_sb = pool.tile([C, NLOC], bf16, tag="mg")
            for a, b in ((0, 2 * SUP), (2 * SUP, 8 * SUP), (8 * SUP, NLOC)):
                nc.sync.dma_start(out=xt_sb[:, a:b], in_=xt[:, a:b])
                nc.sync.dma_start(out=mg_sb[:, a:b], in_=mg[:, a:b])
            o_sb = pool.tile([C, NLOC], bf16, tag="o")
            pieces = ((0, 5), (5, 10), (10, len(sup_spans)))
            for ti, (s0, sl) in enumerate(sup_spans):
                cols = slice(s0, s0 + sl)
                ps = psum.tile([C, SUP], f32, tag="a")
                nc.tensor.matmul(
                    out=ps[:, 0:sl], lhsT=wa_sb[:], rhs=xt_sb[:, cols],
                    start=True, stop=True,
                )
                s_sb = psum.tile([C, SUP], f32, tag="s")
                nc.vector.tensor_add(
                    out=s_sb[:, 0:sl], in0=ps[:, 0:sl], in1=mg_sb[:, cols]
                )
                nc.scalar.activation(
                    out=o_sb[:, cols], in_=s_sb[:, 0:sl],
                    func=mybir.ActivationFunctionType.Relu,
                )
                for p0, p1 in pieces:
                    if ti == p1 - 1:
                        lo = sup_spans[p0][0]
                        hi = sup_spans[p1 - 1][0] + sup_spans[p1 - 1][1]
                        nc.sync.dma_start(
                            out=osh[:, lo:hi], in_=o_sb[:, lo:hi]
                        )
    nc.compile()
    return nc


# ---------------------------------------------------------------------------
# host glue
# ---------------------------------------------------------------------------

def _numpy_fallback(x, edge_index, W, b):
    src, dst = edge_index[0], edge_index[1]
    V1 = W[:, :C] - W[:, C:]
    V2 = W[:, C:]
    A = x @ V1.T + b
    g = x @ V2.T
    out = np.full((x.shape[0], C), -np.inf, dtype=np.float32)
    msg = np.maximum(A[dst] + g[src], 0.0)
    np.maximum.at(out, dst, msg)
    return np.where(np.isneginf(out), 0.0, out).astype(np.float32)


def _run_spmd(nc, in_maps):
    # the shared axon device occasionally reports a transient
    # NRT_EXEC_UNIT_UNRECOVERABLE on a cold first launch; retry once
    import time
    from concourse.bass_utils import run_bass_kernel_spmd

    try:
        return run_bass_kernel_spmd(nc, in_maps, core_ids=list(range(N_CORES)))
    except Exception:
        time.sleep(10.0)
        return run_bass_kernel_spmd(nc, in_maps, core_ids=list(range(N_CORES)))


def kernel(x, edge_index, edge_attr, W, b):
    x = np.ascontiguousarray(x, dtype=np.float32)
    edge_index = np.ascontiguousarray(edge_index, dtype=np.int32)
    W = np.ascontiguousarray(W, dtype=np.float32)
    b = np.ascontiguousarray(b, dtype=np.float32)

    expected_dst = np.repeat(np.arange(N_NODES, dtype=np.int32), DEG)
    if (
        x.shape != (N_NODES, C)
        or edge_index.shape != (2, N_NODES * DEG)
        or not np.array_equal(edge_index[1], expected_dst)
        or edge_index[0].min() < 0
        or edge_index[0].max() >= N_NODES
    ):
        return _numpy_fallback(x, edge_index, W, b)

    src = edge_index[0].astype(np.int64)
    dst = edge_index[1].astype(np.int64)

    ek = edge_index.tobytes()
    if _cache.get("edge_key") != hash(ek):
        _cache["cfg"], _cache["host"] = _host_prep(src, dst)
        _cache["edge_key"] = hash(ek)
    cfg, host = _cache["cfg"], _cache["host"]
    if _cache.get("gather_cfg") != cfg:
        _cache["gather"] = _build_gather(cfg)
        _cache["gather_cfg"] = cfg
    if "dense" not in _cache:
        _cache["dense"] = _build_dense()

    W1, W2 = W[:, :C], W[:, C:]
    # wlo/whi: lhsT columns p -> channel p%32 (+32)
    wlo = np.ascontiguousarray(W2[np.tile(np.arange(32), 4)].T).astype(BF16)
    whi = np.ascontiguousarray(W2[np.tile(np.arange(32, 64), 4)].T).astype(BF16)

    xb = x.astype(BF16)
    in1 = []
    for c in range(N_CORES):
        xt = np.zeros((C, NLOC), dtype=BF16)
        xt[:, :NSH] = xb[c * NSH : (c + 1) * NSH].T
        in1.append(
            {"xt": xt, "wlo": wlo, "whi": whi, "idx": host["idx"][c]}
        )
    r1 = _run_spmd(_cache["gather"], in1)

    # decode per-core partials and merge (max) on host
    rtot = host["rtot"]
    rankb = host["rankb"]
    mfull = np.full((N_NODES, C), -np.inf, dtype=np.float32)
    for c in range(N_CORES):
        part = (
            r1.results[c]["osh"].reshape(128, rtot, 2).astype(np.float32)
        )
        nodes, ks = host["core_nodes"][c]
        ss = host["s_nc"][c, nodes]
        rr = rankb[ks] + host["q_nc"][c, nodes]
        vals = np.empty((len(nodes), C), dtype=np.float32)
        for s in range(NSTR):
            sel = ss == s
            if not sel.any():
                continue
            blk = part[32 * s : 32 * (s + 1), rr[sel], :]  # [32, m, 2]
            vals[sel, 0:32] = blk[:, :, 0].T
            vals[sel, 32:64] = blk[:, :, 1].T
        mfull[nodes] = np.maximum(mfull[nodes], vals)

    # ---- Launch 2: node-sharded A + max, relu ----
    wa = np.concatenate(
        [(W1 - W2).T, b.reshape(1, C)], axis=0
    ).astype(BF16)
    in2 = []
    for c in range(N_CORES):
        xt2 = np.zeros((C + 1, NLOC), dtype=BF16)
        xt2[:C, :NSH] = xb[c * NSH : (c + 1) * NSH].T
        xt2[C] = 1.0
        mgc = np.zeros((C, NLOC), dtype=BF16)
        mgc[:, :NSH] = mfull[c * NSH : (c + 1) * NSH].T.astype(BF16)
        in2.append({"xt": xt2, "wa": wa, "mg": mgc})
    r2 = _run_spmd(_cache["dense"], in2)

    out = np.empty((N_CORES, NSH, C), dtype=np.float32)
    for c in range(N_CORES):
        out[c] = r2.results[c]["osh"][:, :NSH].T.astype(np.float32)
    _cache["last_results"] = (r1, r2)
    return out.reshape(N_NODES, C)


# revision 5
# speedup vs baseline: 2.1462x; 1.0996x over previous
"""EdgeConv (PyG, aggr='max') Trainium2 kernel, 8-core SPMD.

Math: out_i = max_{e: dst(e)=i} relu(x_i @ W1.T + (x_src(e) - x_i) @ W2.T + b)
with W = [W1 | W2].  Rewriting:
    msg_e = A_i + g_src(e),  A = x @ (W1-W2).T + b,  g = x @ W2.T
Since A_i is constant within segment i and relu is monotone:
    out_i = relu(A_i + max_e g_src(e))

Sharding: edges are partitioned across cores BY SOURCE RANGE (core c owns
srcs [6250c, 6250(c+1))), so each core's entire gather table is its own
locally-computed g-shard kept in SBUF -- no HBM gather at all.  The table is
channel-paired int32 [128, 6400]: partition p holds channels (p%32, p%32+32)
packed as 2xbf16, replicated over 4 independent 32-partition "streams".  A
single GPSIMD ap_gather column then fetches a full 64-channel row for 4
different edges at once (one per stream) at ~0.35 ns/edge -- 4x cheaper than
the DMA-descriptor path and on an otherwise idle engine.

Each core's destination nodes are grouped by their per-core edge count k
(host-side reorder); the segment max becomes regular k-window max trees on
DVE over the gathered columns.  Per-core partial maxes [128, R] are written
out; the host un-permutes and np.maximum-merges them across cores (pure
unshard glue).  A second small node-sharded launch computes
relu(A + merged_max) and writes the final bf16 output.

Launch 1 (gather): xt/wlo/whi -> PE builds the paired table; 4 ap_gather
chunks of ~6400 columns pipeline with DVE max-tree spans; osh partials.
Launch 2 (dense): 13 supertile matmuls for A, DVE adds the merged max,
ACT applies relu.
"""

import math

import numpy as np
import ml_dtypes

BF16 = ml_dtypes.bfloat16

N_NODES = 50000
DEG = 16
C = 64
N_CORES = 8
NSH = N_NODES // N_CORES  # 6250 src nodes per core
NLOC = 6400  # padded local node count (table columns / L2 shard columns)
NSTR = 4  # gather streams (32 partitions each)
KMAX = DEG
SUP = 512  # supertile columns (one PSUM bank)
KC_TARGET = NLOC  # gather chunk columns ~ table size (cost floor)

_cache = {}


# ---------------------------------------------------------------------------
# host-side layout
# ---------------------------------------------------------------------------

def _host_prep(src, dst):
    """Compute the SPMD-uniform column layout from the actual edge list.

    Returns cfg (hashable, shapes for the kernel builder) and per-core host
    data (idx arrays, decode maps).
    """
    E = src.shape[0]
    core = (src // NSH).astype(np.int64)
    n64 = dst.astype(np.int64)
    kmat = np.bincount(n64 * N_CORES + core, minlength=N_NODES * N_CORES)
    kmat = kmat.reshape(N_NODES, N_CORES)  # [N, 8] per-(node, core) edge count

    # group nodes by k per core; round-robin nodes over 4 streams
    s_nc = np.zeros((N_CORES, N_NODES), np.int64)
    q_nc = np.zeros((N_CORES, N_NODES), np.int64)
    caps = np.zeros(KMAX + 1, np.int64)  # per-(k, stream) node capacity
    core_nodes = []  # per core: (nodes, k of nodes)
    for c in range(N_CORES):
        kc = kmat[:, c]
        nz = np.nonzero(kc)[0]
        core_nodes.append((nz, kc[nz]))
        for k in range(1, KMAX + 1):
            nodes = nz[kc[nz] == k]
            m = len(nodes)
            if m == 0:
                continue
            r = np.arange(m)
            s_nc[c, nodes] = r % NSTR
            q_nc[c, nodes] = r // NSTR
            caps[k] = max(caps[k], -(-m // NSTR))
    # pad caps so every k-group is 16-column aligned; chunk boundaries are
    # additionally restricted to 128-aligned node starts below (the GPSIMD
    # gather needs 4B-aligned index slices)
    for k in range(1, KMAX + 1):
        if caps[k]:
            step = 16 // math.gcd(k, 16)
            caps[k] = -(-caps[k] // step) * step
    offk = np.zeros(KMAX + 2, np.int64)
    rankb = np.zeros(KMAX + 2, np.int64)
    for k in range(1, KMAX + 1):
        offk[k + 1] = offk[k] + caps[k] * k
        rankb[k + 1] = rankb[k] + caps[k]
    ctot = int(offk[KMAX + 1])
    rtot = int(rankb[KMAX + 1])

    # node-start columns in rank order (for chunk boundary search)
    starts = np.concatenate(
        [offk[k] + np.arange(caps[k]) * k for k in range(1, KMAX + 1) if caps[k]]
    )
    # chunk boundaries: near equal quarters, node-aligned and 16-aligned
    nch = max(1, int(round(ctot / KC_TARGET)))
    aligned = starts[starts % 128 == 0]
    bounds = [0]
    for i in range(1, nch):
        t = ctot * i // nch
        j = int(np.argmin(np.abs(aligned - t)))
        b = int(aligned[j])
        if b <= bounds[-1]:
            continue
        bounds.append(b)
    bounds.append(ctot)
    # chunk rank bounds
    rb = [int(np.searchsorted(starts, b)) for b in bounds]
    chunks = []
    for i in range(len(bounds) - 1):
        chunks.append((bounds[i], bounds[i + 1] - bounds[i], rb[i], rb[i + 1]))

    # reduce spans: (chunk_idx, col0, n_nodes, k, rank0), split at chunk bounds
    spans = []
    for k in range(1, KMAX + 1):
        if not caps[k]:
            continue
        g0, g1 = int(offk[k]), int(offk[k + 1])
        for ci, (b0, kc_, _, _) in enumerate(chunks):
            lo, hi = max(g0, b0), min(g1, b0 + kc_)
            if lo >= hi:
                continue
            spans.append(
                (ci, lo, (hi - lo) // k, k, int(rankb[k] + (lo - g0) // k))
            )

    # per-edge column assignment
    key = core * N_NODES + n64
    order = np.argsort(key, kind="stable")
    sk = key[order]
    first_new = np.r_[True, sk[1:] != sk[:-1]]
    run_id = np.cumsum(first_new) - 1
    run_start = np.nonzero(first_new)[0]
    j_sorted = np.arange(E) - run_start[run_id]
    occ = np.empty(E, np.int64)
    occ[order] = j_sorted
    k_e = kmat[n64, core]
    col_e = offk[k_e] + q_nc[core, n64] * k_e + occ
    s_e = s_nc[core, n64]
    ls_e = (src - core * NSH).astype(np.int64)

    idxs = np.zeros((N_CORES, NSTR, ctot), np.int16)
    idxs[core, s_e, col_e] = ls_e

    # wrapped idx layout [128, ctot//16]: group g (partitions 16g..16g+16)
    # carries stream g//2's list, element j at [16g + j%16, j//16]
    idx_wrapped = np.empty((N_CORES, 128, ctot // 16), np.int16)
    for c in range(N_CORES):
        a = idxs[c].reshape(NSTR, ctot // 16, 16)
        for g in range(8):
            idx_wrapped[c, 16 * g : 16 * (g + 1), :] = a[g // 2].T

    cfg = (ctot, rtot, tuple(chunks), tuple(spans))
    host = {
        "idx": idx_wrapped,
        "core_nodes": core_nodes,
        "s_nc": s_nc,
        "q_nc": q_nc,
        "rankb": rankb,
        "rtot": rtot,
    }
    return cfg, host


# ---------------------------------------------------------------------------
# launch 1: src-sharded gather + per-core segment max partials
# ---------------------------------------------------------------------------

def _build_gather(cfg):
    import concourse.bacc as bacc
    import concourse.mybir as mybir
    from concourse.tile import TileContext

    ctot, rtot, chunks, spans = cfg
    kc_max = max(kc for _, kc, _, _ in chunks)

    nc = bacc.Bacc("TRN2", target_bir_lowering=False, debug=False)
    f32 = mybir.dt.float32
    bf16 = mybir.dt.bfloat16
    i32 = mybir.dt.int32
    i16 = mybir.dt.int16
    mx = mybir.AluOpType.max

    xt = nc.dram_tensor("xt", [C, NLOC], bf16, kind="ExternalInput")
    wlo = nc.dram_tensor("wlo", [C, 128], bf16, kind="ExternalInput")
    whi = nc.dram_tensor("whi", [C, 128], bf16, kind="ExternalInput")
    idx = nc.dram_tensor("idx", [128, ctot // 16], i16, kind="ExternalInput")
    osh = nc.dram_tensor("osh", [128, 2 * rtot], bf16, kind="ExternalOutput")

    nsup = NLOC // SUP  # 12.5 -> handle tail below
    sup_spans = [(i * SUP, SUP) for i in range(NLOC // SUP)]
    if NLOC % SUP:
        sup_spans.append((NLOC // SUP * SUP, NLOC % SUP))

    with TileContext(nc) as tc:
        with (
            tc.tile_pool(name="const", bufs=1) as cpool,
            tc.tile_pool(name="sbuf", bufs=1) as pool,
            tc.tile_pool(name="gat", bufs=3) as gpool,
            tc.tile_pool(name="psum", bufs=4, space="PSUM") as psum,
        ):
            # small loads first: weights + the gather index list
            wlo_sb = cpool.tile([C, 128], bf16)
            nc.sync.dma_start(out=wlo_sb[:], in_=wlo[:])
            whi_sb = cpool.tile([C, 128], bf16)
            nc.sync.dma_start(out=whi_sb[:], in_=whi[:])
            idx_sb = pool.tile([128, ctot // 16], i16, tag="idx")
            nc.sync.dma_start(out=idx_sb[:], in_=idx[:])
            xt_sb = pool.tile([C, NLOC], bf16, tag="xt")
            for a, b in ((0, 2 * SUP), (2 * SUP, 8 * SUP), (8 * SUP, NLOC)):
                nc.sync.dma_start(out=xt_sb[:, a:b], in_=xt[:, a:b])

            # paired g table: int32[p, n] = (g[n, p%32] , g[n, p%32+32])
            tbl = pool.tile([128, NLOC], i32, tag="tbl")
            tbl_bf = tbl[:].bitcast(bf16).rearrange("p (n t) -> p n t", t=2)
            for ti, (s0, sl) in enumerate(sup_spans):
                cols = slice(s0, s0 + sl)
                ps_lo = psum.tile([128, SUP], f32, tag="plo")
                nc.tensor.matmul(
                    out=ps_lo[:, 0:sl], lhsT=wlo_sb[:], rhs=xt_sb[:, cols],
                    start=True, stop=True,
                )
                nc.scalar.copy(out=tbl_bf[:, cols, 0], in_=ps_lo[:, 0:sl])
                ps_hi = psum.tile([128, SUP], f32, tag="phi")
                nc.tensor.matmul(
                    out=ps_hi[:, 0:sl], lhsT=whi_sb[:], rhs=xt_sb[:, cols],
                    start=True, stop=True,
                )
                nc.vector.tensor_copy(out=tbl_bf[:, cols, 1], in_=ps_hi[:, 0:sl])

            osh_buf = pool.tile([128, rtot], i32, tag="oshb")
            osh_bf = osh_buf[:].bitcast(bf16).rearrange("p (n t) -> p n t", t=2)

            for ci, (b0, kc, cr0, cr1) in enumerate(chunks):
                g = gpool.tile([128, kc_max], i32, tag="g")
                nc.gpsimd.ap_gather(
                    out_ap=g[:, 0:kc].rearrange("p (n d) -> p n d", d=1),
                    in_ap=tbl[:].rearrange("p (n d) -> p n d", d=1),
                    idxs_ap=idx_sb[:, b0 // 16 : (b0 + kc) // 16],
                    channels=128, num_elems=NLOC, d=1, num_idxs=kc,
                )
                g_bf = g[:].bitcast(bf16)
                for sci, col0, nn, k, r0 in spans:
                    if sci != ci:
                        continue
                    l0 = col0 - b0
                    v = g_bf[:, 2 * l0 : 2 * (l0 + nn * k)].rearrange(
                        "p (n k t) -> p n k t", k=k, t=2
                    )
                    dst = osh_bf[:, r0 : r0 + nn, :]
                    if k == 1:
                        nc.vector.tensor_copy(out=dst, in_=v[:, :, 0, :])
                        continue
                    j = k
                    while j > 2:
                        if j % 2:
                            nc.vector.tensor_tensor(
                                out=v[:, :, 0, :], in0=v[:, :, 0, :],
                                in1=v[:, :, j - 1, :], op=mx,
                            )
                            j -= 1
                        m = j // 2
                        if j > 2:
                            nc.vector.tensor_tensor(
                                out=v[:, :, 0:m, :], in0=v[:, :, 0:m, :],
                                in1=v[:, :, m : 2 * m, :], op=mx,
                            )
                            j = m
                    nc.vector.tensor_tensor(
                        out=dst, in0=v[:, :, 0, :], in1=v[:, :, 1, :], op=mx
                    )
                if cr1 > cr0:
                    nc.sync.dma_start(
                        out=osh[:, 2 * cr0 : 2 * cr1],
                        in_=osh_buf[:].bitcast(bf16)[:, 2 * cr0 : 2 * cr1],
                    )
    nc.compile()
    return nc


# ---------------------------------------------------------------------------
# launch 2: node-sharded A + merged max, relu
# ---------------------------------------------------------------------------

def _build_dense():
    import concourse.bacc as bacc
    import concourse.mybir as mybir
    from concourse.tile import TileContext

    nc = bacc.Bacc("TRN2", target_bir_lowering=False, debug=False)
    f32 = mybir.dt.float32
    bf16 = mybir.dt.bfloat16

    xt = nc.dram_tensor("xt", [C + 1, NLOC], bf16, kind="ExternalInput")
    wa = nc.dram_tensor("wa", [C + 1, C], bf16, kind="ExternalInput")
    mg = nc.dram_tensor("mg", [C, NLOC], bf16, kind="ExternalInput")
    osh = nc.dram_tensor("osh", [C, NLOC], bf16, kind="ExternalOutput")

    sup_spans = [(i * SUP, SUP) for i in range(NLOC // SUP)]
    if NLOC % SUP:
        sup_spans.append((NLOC // SUP * SUP, NLOC % SUP))

    with TileContext(nc) as tc:
        with (
            tc.tile_pool(name="const", bufs=1) as cpool,
            tc.tile_pool(name="sbuf", bufs=1) as pool,
            tc.tile_pool(name="psum", bufs=4, space="PSUM") as psum,
        ):
            wa_sb = cpool.tile([C + 1, C], bf16)
            nc.sync.dma_start(out=wa_sb[:], in_=wa[:])
            # preload the ACT function table while inputs stream in
            warm = cpool.tile([1, 2], f32)
            nc.vector.memset(warm[:], 0.0)
            warm2 = cpool.tile([1, 2], f32)
            nc.scalar.activation(
                out=warm2[:], in_=warm[:],
                func=mybir.ActivationFunctionType.Relu,
            )
            xt_sb = pool.tile([C + 1, NLOC], bf16, tag="xt")
            mg_sb = pool.tile([C, NLOC], bf16, tag="mg")
            for a, b in ((0, 2 * SUP), (2 * SUP, 8 * SUP), (8 * SUP, NLOC)):
                nc.sync.dma_start(out=xt_sb[:, a:b], in_=xt[:, a:b])
                nc.sync.dma_start(out=mg_sb[:, a:b], in_=mg[:, a:b])
            o_sb = pool.tile([C, NLOC], bf16, tag="o")
            pieces = ((0, 5), (5, 10), (10, len(sup_spans)))
            for ti, (s0, sl) in enumerate(sup_spans):
                cols = slice(s0, s0 + sl)
                ps = psum.tile([C, SUP], f32, tag="a")
                nc.tensor.matmul(
                    out=ps[:, 0:sl], lhsT=wa_sb[:], rhs=xt_sb[:, cols],
                    start=True, stop=True,
                )
                s_sb = psum.tile([C, SUP], f32, tag="s")
                nc.vector.tensor_add(
                    out=s_sb[:, 0:sl], in0=ps[:, 0:sl], in1=mg_sb[:, cols]
                )
                nc.scalar.activation(
                    out=o_sb[:, cols], in_=s_sb[:, 0:sl],
                    func=mybir.ActivationFunctionType.Relu,
                )
                for p0, p1 in pieces:
                    if ti == p1 - 1:
                        lo = sup_spans[p0][0]
                        hi = sup_spans[p1 - 1][0] + sup_spans[p1 - 1][1]
                        nc.sync.dma_start(
                            out=osh[:, lo:hi], in_=o_sb[:, lo:hi]
                        )
    nc.compile()
    return nc


# ---------------------------------------------------------------------------
# host glue
# ---------------------------------------------------------------------------

def _numpy_fallback(x, edge_index, W, b):
    src, dst = edge_index[0], edge_index[1]
    V1 = W[:, :C] - W[:, C:]
    V2 = W[:, C:]
    A = x @ V1.T + b
    g = x @ V2.T
    out = np.full((x.shape[0], C), -np.inf, dtype=np.float32)
    msg = np.maximum(A[dst] + g[src], 0.0)
    np.maximum.at(out, dst, msg)
    return np.where(np.isneginf(out), 0.0, out).astype(np.float32)


def _run_spmd(nc, in_maps):
    # the shared axon device occasionally reports a transient
    # NRT_EXEC_UNIT_UNRECOVERABLE on a cold first launch; retry once
    import time
    from concourse.bass_utils import run_bass_kernel_spmd

    try:
        return run_bass_kernel_spmd(nc, in_maps, core_ids=list(range(N_CORES)))
    except Exception:
        time.sleep(10.0)
        return run_bass_kernel_spmd(nc, in_maps, core_ids=list(range(N_CORES)))


def kernel(x, edge_index, edge_attr, W, b):
    x = np.ascontiguousarray(x, dtype=np.float32)
    edge_index = np.ascontiguousarray(edge_index, dtype=np.int32)
    W = np.ascontiguousarray(W, dtype=np.float32)
    b = np.ascontiguousarray(b, dtype=np.float32)

    expected_dst = np.repeat(np.arange(N_NODES, dtype=np.int32), DEG)
    if (
        x.shape != (N_NODES, C)
        or edge_index.shape != (2, N_NODES * DEG)
        or not np.array_equal(edge_index[1], expected_dst)
        or edge_index[0].min() < 0
        or edge_index[0].max() >= N_NODES
    ):
        return _numpy_fallback(x, edge_index, W, b)

    src = edge_index[0].astype(np.int64)
    dst = edge_index[1].astype(np.int64)

    ek = edge_index.tobytes()
    if _cache.get("edge_key") != hash(ek):
        _cache["cfg"], _cache["host"] = _host_prep(src, dst)
        _cache["edge_key"] = hash(ek)
    cfg, host = _cache["cfg"], _cache["host"]
    if _cache.get("gather_cfg") != cfg:
        _cache["gather"] = _build_gather(cfg)
        _cache["gather_cfg"] = cfg
    if "dense" not in _cache:
        _cache["dense"] = _build_dense()

    W1, W2 = W[:, :C], W[:, C:]
    # wlo/whi: lhsT columns p -> channel p%32 (+32)
    wlo = np.ascontiguousarray(W2[np.tile(np.arange(32), 4)].T).astype(BF16)
    whi = np.ascontiguousarray(W2[np.tile(np.arange(32, 64), 4)].T).astype(BF16)

    xb = x.astype(BF16)
    in1 = []
    for c in range(N_CORES):
        xt = np.zeros((C, NLOC), dtype=BF16)
        xt[:, :NSH] = xb[c * NSH : (c + 1) * NSH].T
        in1.append(
            {"xt": xt, "wlo": wlo, "whi": whi, "idx": host["idx"][c]}
        )
    r1 = _run_spmd(_cache["gather"], in1)

    # decode per-core partials and merge (max) on host
    rtot = host["rtot"]
    rankb = host["rankb"]
    mfull = np.full((N_NODES, C), -np.inf, dtype=np.float32)
    for c in range(N_CORES):
        part = (
            r1.results[c]["osh"].reshape(128, rtot, 2).astype(np.float32)
        )
        nodes, ks = host["core_nodes"][c]
        ss = host["s_nc"][c, nodes]
        rr = rankb[ks] + host["q_nc"][c, nodes]
        vals = np.empty((len(nodes), C), dtype=np.float32)
        for s in range(NSTR):
            sel = ss == s
            if not sel.any():
                continue
            blk = part[32 * s : 32 * (s + 1), rr[sel], :]  # [32, m, 2]
            vals[sel, 0:32] = blk[:, :, 0].T
            vals[sel, 32:64] = blk[:, :, 1].T
        mfull[nodes] = np.maximum(mfull[nodes], vals)

    # ---- Launch 2: node-sharded A + max, relu ----
    wa = np.concatenate(
        [(W1 - W2).T, b.reshape(1, C)], axis=0
    ).astype(BF16)
    in2 = []
    for c in range(N_CORES):
        xt2 = np.zeros((C + 1, NLOC), dtype=BF16)
        xt2[:C, :NSH] = xb[c * NSH : (c + 1) * NSH].T
        xt2[C] = 1.0
        mgc = np.zeros((C, NLOC), dtype=BF16)
        mgc[:, :NSH] = mfull[c * NSH : (c + 1) * NSH].T.astype(BF16)
        in2.append({"xt": xt2, "wa": wa, "mg": mgc})
    r2 = _run_spmd(_cache["dense"], in2)

    out = np.empty((N_CORES, NSH, C), dtype=np.float32)
    for c in range(N_CORES):
        out[c] = r2.results[c]["osh"][:, :NSH].T.astype(np.float32)
    _cache["last_results"] = (r1, r2)
    return out.reshape(N_NODES, C)


# revision 9
# speedup vs baseline: 2.1678x; 1.0100x over previous
"""EdgeConv (PyG, aggr='max') Trainium2 kernel, 8-core SPMD.

Math: out_i = max_{e: dst(e)=i} relu(x_i @ W1.T + (x_src(e) - x_i) @ W2.T + b)
with W = [W1 | W2].  Rewriting:
    msg_e = A_i + g_src(e),  A = x @ (W1-W2).T + b,  g = x @ W2.T
Since A_i is constant within segment i and relu is monotone:
    out_i = relu(A_i + max_e g_src(e))

Sharding: edges are partitioned across cores BY SOURCE RANGE (core c owns
srcs [6250c, 6250(c+1))), so each core's entire gather table is its own
locally-computed g-shard kept in SBUF -- no HBM gather at all.  The table is
channel-paired int32 [128, 6400]: partition p holds channels (p%32, p%32+32)
packed as 2xbf16, replicated over 4 independent 32-partition "streams".  A
single GPSIMD ap_gather column then fetches a full 64-channel row for 4
different edges at once (one per stream) at ~0.35 ns/edge -- 4x cheaper than
the DMA-descriptor path and on an otherwise idle engine.

Each core's destination nodes are grouped by their per-core edge count k
(host-side reorder); the segment max becomes regular k-window max trees on
DVE over the gathered columns.  Per-core partial maxes [128, R] are written
out; the host un-permutes and np.maximum-merges them across cores (pure
unshard glue).  A second small node-sharded launch computes
relu(A + merged_max) and writes the final bf16 output.

Launch 1 (gather): xt/wlo/whi -> PE builds the paired table; 4 ap_gather
chunks of ~6400 columns pipeline with DVE max-tree spans; osh partials.
Launch 2 (dense): 13 supertile matmuls for A, DVE adds the merged max,
ACT applies relu.
"""

import math

import numpy as np
import ml_dtypes

BF16 = ml_dtypes.bfloat16

N_NODES = 50000
DEG = 16
C = 64
N_CORES = 8
NSH = N_NODES // N_CORES  # 6250 src nodes per core
NLOC = 6400  # padded local node count (table columns / L2 shard columns)
NSTR = 4  # gather streams (32 partitions each)
KMAX = DEG
SUP = 512  # supertile columns (one PSUM bank)
KC_TARGET = NLOC  # gather chunk columns ~ table size (cost floor)

_cache = {}


# ---------------------------------------------------------------------------
# host-side layout
# ---------------------------------------------------------------------------

def _host_prep(src, dst):
    """Compute the SPMD-uniform column layout from the actual edge list.

    Returns cfg (hashable, shapes for the kernel builder) and per-core host
    data (idx arrays, decode maps).
    """
    E = src.shape[0]
    core = (src // NSH).astype(np.int64)
    n64 = dst.astype(np.int64)
    kmat = np.bincount(n64 * N_CORES + core, minlength=N_NODES * N_CORES)
    kmat = kmat.reshape(N_NODES, N_CORES)  # [N, 8] per-(node, core) edge count

    # group nodes by k per core; round-robin nodes over 4 streams
    s_nc = np.zeros((N_CORES, N_NODES), np.int64)
    q_nc = np.zeros((N_CORES, N_NODES), np.int64)
    caps = np.zeros(KMAX + 1, np.int64)  # per-(k, stream) node capacity
    core_nodes = []  # per core: (nodes, k of nodes)
    for c in range(N_CORES):
        kc = kmat[:, c]
        nz = np.nonzero(kc)[0]
        core_nodes.append((nz, kc[nz]))
        for k in range(1, KMAX + 1):
            nodes = nz[kc[nz] == k]
            m = len(nodes)
            if m == 0:
                continue
            r = np.arange(m)
            s_nc[c, nodes] = r % NSTR
            q_nc[c, nodes] = r // NSTR
            caps[k] = max(caps[k], -(-m // NSTR))
    # pad caps so every k-group is 16-column aligned; chunk boundaries are
    # additionally restricted to 128-aligned node starts below (the GPSIMD
    # gather needs 4B-aligned index slices)
    for k in range(1, KMAX + 1):
        if caps[k]:
            step = 16 // math.gcd(k, 16)
            caps[k] = -(-caps[k] // step) * step
    # group layout order: k=2 last (fewest DVE ops per column, so the
    # post-last-gather reduce trail is shortest)
    order = [1] + list(range(3, KMAX + 1)) + [2]
    offk = np.zeros(KMAX + 1, np.int64)
    rankb = np.zeros(KMAX + 1, np.int64)
    off = rk = 0
    for k in order:
        offk[k] = off
        rankb[k] = rk
        off += caps[k] * k
        rk += caps[k]
    ctot = int(off)
    rtot = int(rk)

    # node-start columns in column order (for chunk boundary search)
    starts = np.concatenate(
        [offk[k] + np.arange(caps[k]) * k for k in order if caps[k]]
    )
    # chunk boundaries: near equal quarters, node-aligned and 16-aligned
    nch = max(1, int(round(ctot / KC_TARGET)))
    aligned = starts[starts % 128 == 0]
    bounds = [0]
    for i in range(1, nch):
        t = ctot * i // nch
        j = int(np.argmin(np.abs(aligned - t)))
        b = int(aligned[j])
        if b <= bounds[-1]:
            continue
        bounds.append(b)
    bounds.append(ctot)
    # chunk rank bounds
    rb = [int(np.searchsorted(starts, b)) for b in bounds]
    chunks = []
    for i in range(len(bounds) - 1):
        chunks.append((bounds[i], bounds[i + 1] - bounds[i], rb[i], rb[i + 1]))

    # reduce spans: (chunk_idx, col0, n_nodes, k, rank0), split at chunk bounds
    spans = []
    for k in order:
        if not caps[k]:
            continue
        g0, g1 = int(offk[k]), int(offk[k] + caps[k] * k)
        for ci, (b0, kc_, _, _) in enumerate(chunks):
            lo, hi = max(g0, b0), min(g1, b0 + kc_)
            if lo >= hi:
                continue
            spans.append(
                (ci, lo, (hi - lo) // k, k, int(rankb[k] + (lo - g0) // k))
            )

    # per-edge column assignment
    key = core * N_NODES + n64
    order = np.argsort(key, kind="stable")
    sk = key[order]
    first_new = np.r_[True, sk[1:] != sk[:-1]]
    run_id = np.cumsum(first_new) - 1
    run_start = np.nonzero(first_new)[0]
    j_sorted = np.arange(E) - run_start[run_id]
    occ = np.empty(E, np.int64)
    occ[order] = j_sorted
    k_e = kmat[n64, core]
    col_e = offk[k_e] + q_nc[core, n64] * k_e + occ
    s_e = s_nc[core, n64]
    ls_e = (src - core * NSH).astype(np.int64)

    idxs = np.zeros((N_CORES, NSTR, ctot), np.int16)
    idxs[core, s_e, col_e] = ls_e

    # wrapped idx layout [128, ctot//16]: group g (partitions 16g..16g+16)
    # carries stream g//2's list, element j at [16g + j%16, j//16]
    idx_wrapped = np.empty((N_CORES, 128, ctot // 16), np.int16)
    for c in range(N_CORES):
        a = idxs[c].reshape(NSTR, ctot // 16, 16)
        for g in range(8):
            idx_wrapped[c, 16 * g : 16 * (g + 1), :] = a[g // 2].T

    cfg = (ctot, rtot, tuple(chunks), tuple(spans))
    host = {
        "idx": idx_wrapped,
        "core_nodes": core_nodes,
        "s_nc": s_nc,
        "q_nc": q_nc,
        "rankb": rankb,
        "rtot": rtot,
    }
    return cfg, host


# ---------------------------------------------------------------------------
# launch 1: src-sharded gather + per-core segment max partials
# ---------------------------------------------------------------------------

def _build_gather(cfg):
    import concourse.bacc as bacc
    import concourse.mybir as mybir
    from concourse.tile import TileContext

    ctot, rtot, chunks, spans = cfg
    kc_max = max(kc for _, kc, _, _ in chunks)

    nc = bacc.Bacc("TRN2", target_bir_lowering=False, debug=False)
    f32 = mybir.dt.float32
    bf16 = mybir.dt.bfloat16
    i32 = mybir.dt.int32
    i16 = mybir.dt.int16
    mx = mybir.AluOpType.max

    xt = nc.dram_tensor("xt", [C, NLOC], bf16, kind="ExternalInput")
    wlo = nc.dram_tensor("wlo", [C, 128], bf16, kind="ExternalInput")
    whi = nc.dram_tensor("whi", [C, 128], bf16, kind="ExternalInput")
    idx = nc.dram_tensor("idx", [128, ctot // 16], i16, kind="ExternalInput")
    osh = nc.dram_tensor("osh", [128, 2 * rtot], bf16, kind="ExternalOutput")

    nsup = NLOC // SUP  # 12.5 -> handle tail below
    sup_spans = [(i * SUP, SUP) for i in range(NLOC // SUP)]
    if NLOC % SUP:
        sup_spans.append((NLOC // SUP * SUP, NLOC % SUP))

    with TileContext(nc) as tc:
        with (
            tc.tile_pool(name="const", bufs=1) as cpool,
            tc.tile_pool(name="sbuf", bufs=1) as pool,
            tc.tile_pool(name="gat", bufs=3) as gpool,
            tc.tile_pool(name="psum", bufs=4, space="PSUM") as psum,
        ):
            # small loads first: weights + the gather index list
            wlo_sb = cpool.tile([C, 128], bf16)
            nc.sync.dma_start(out=wlo_sb[:], in_=wlo[:])
            whi_sb = cpool.tile([C, 128], bf16)
            nc.sync.dma_start(out=whi_sb[:], in_=whi[:])
            idx_sb = pool.tile([128, ctot // 16], i16, tag="idx")
            nc.sync.dma_start(out=idx_sb[:], in_=idx[:])
            xt_sb = pool.tile([C, NLOC], bf16, tag="xt")
            for a, b in ((0, 2 * SUP), (2 * SUP, 8 * SUP), (8 * SUP, NLOC)):
                nc.sync.dma_start(out=xt_sb[:, a:b], in_=xt[:, a:b])

            # paired g table: int32[p, n] = (g[n, p%32] , g[n, p%32+32])
            tbl = pool.tile([128, NLOC], i32, tag="tbl")
            tbl_bf = tbl[:].bitcast(bf16).rearrange("p (n t) -> p n t", t=2)
            for ti, (s0, sl) in enumerate(sup_spans):
                cols = slice(s0, s0 + sl)
                ps_lo = psum.tile([128, SUP], f32, tag="plo")
                nc.tensor.matmul(
                    out=ps_lo[:, 0:sl], lhsT=wlo_sb[:], rhs=xt_sb[:, cols],
                    start=True, stop=True,
                )
                nc.scalar.copy(out=tbl_bf[:, cols, 0], in_=ps_lo[:, 0:sl])
                ps_hi = psum.tile([128, SUP], f32, tag="phi")
                nc.tensor.matmul(
                    out=ps_hi[:, 0:sl], lhsT=whi_sb[:], rhs=xt_sb[:, cols],
                    start=True, stop=True,
                )
                nc.vector.tensor_copy(out=tbl_bf[:, cols, 1], in_=ps_hi[:, 0:sl])

            osh_buf = pool.tile([128, rtot], i32, tag="oshb")
            osh_bf = osh_buf[:].bitcast(bf16).rearrange("p (n t) -> p n t", t=2)

            for ci, (b0, kc, cr0, cr1) in enumerate(chunks):
                g = gpool.tile([128, kc_max], i32, tag="g")
                nc.gpsimd.ap_gather(
                    out_ap=g[:, 0:kc].rearrange("p (n d) -> p n d", d=1),
                    in_ap=tbl[:].rearrange("p (n d) -> p n d", d=1),
                    idxs_ap=idx_sb[:, b0 // 16 : (b0 + kc) // 16],
                    channels=128, num_elems=NLOC, d=1, num_idxs=kc,
                )
                g_bf = g[:].bitcast(bf16)
                for sci, col0, nn, k, r0 in spans:
                    if sci != ci:
                        continue
                    l0 = col0 - b0
                    v = g_bf[:, 2 * l0 : 2 * (l0 + nn * k)].rearrange(
                        "p (n k t) -> p n k t", k=k, t=2
                    )
                    dst = osh_bf[:, r0 : r0 + nn, :]
                    if k == 1:
                        nc.vector.tensor_copy(out=dst, in_=v[:, :, 0, :])
                        continue
                    j = k
                    while j > 2:
                        if j % 2:
                            nc.vector.tensor_tensor(
                                out=v[:, :, 0, :], in0=v[:, :, 0, :],
                                in1=v[:, :, j - 1, :], op=mx,
                            )
                            j -= 1
                        m = j // 2
                        if j > 2:
                            nc.vector.tensor_tensor(
                                out=v[:, :, 0:m, :], in0=v[:, :, 0:m, :],
                                in1=v[:, :, m : 2 * m, :], op=mx,
                            )
                            j = m
                    nc.vector.tensor_tensor(
                        out=dst, in0=v[:, :, 0, :], in1=v[:, :, 1, :], op=mx
                    )
                if cr1 > cr0:
                    nc.sync.dma_start(
                        out=osh[:, 2 * cr0 : 2 * cr1],
                        in_=osh_buf[:].bitcast(bf16)[:, 2 * cr0 : 2 * cr1],
                    )
    nc.compile()
    return nc


# ---------------------------------------------------------------------------
# launch 2: node-sharded A + merged max, relu
# ---------------------------------------------------------------------------

def _build_dense():
    """relu(A + M) node-sharded.  The merged max M (with bias pre-added) is
    carried as extra rhs rows against an identity block in lhsT, so a single
    matmul yields A + M + b in PSUM and ACT applies the relu directly."""
    import concourse.bacc as bacc
    import concourse.mybir as mybir
    from concourse.tile import TileContext

    nc = bacc.Bacc("TRN2", target_bir_lowering=False, debug=False)
    f32 = mybir.dt.float32
    bf16 = mybir.dt.bfloat16

    # rows 0..63 = x-shard^T, rows 64..127 = (M + b)^T
    xt = nc.dram_tensor("xt", [2 * C, NLOC], bf16, kind="ExternalInput")
    wa = nc.dram_tensor("wa", [2 * C, C], bf16, kind="ExternalInput")
    osh = nc.dram_tensor("osh", [C, NLOC], bf16, kind="ExternalOutput")

    sup_spans = [(i * SUP, SUP) for i in range(NLOC // SUP)]
    if NLOC % SUP:
        sup_spans.append((NLOC // SUP * SUP, NLOC % SUP))

    with TileContext(nc) as tc:
        with (
            tc.tile_pool(name="const", bufs=1) as cpool,
            tc.tile_pool(name="sbuf", bufs=1) as pool,
            tc.tile_pool(name="psum", bufs=4, space="PSUM") as psum,
        ):
            wa_sb = cpool.tile([2 * C, C], bf16)
            nc.sync.dma_start(out=wa_sb[:], in_=wa[:])
            # preload the ACT function table while inputs stream in
            warm = cpool.tile([1, 2], f32)
            nc.vector.memset(warm[:], 0.0)
            warm2 = cpool.tile([1, 2], f32)
            nc.scalar.activation(
                out=warm2[:], in_=warm[:],
                func=mybir.ActivationFunctionType.Relu,
            )
            xt_sb = pool.tile([2 * C, NLOC], bf16, tag="xt")
            for a, b in ((0, SUP), (SUP, 4 * SUP), (4 * SUP, 8 * SUP),
                         (8 * SUP, NLOC)):
                nc.sync.dma_start(out=xt_sb[:, a:b], in_=xt[:, a:b])
            o_sb = pool.tile([C, NLOC], bf16, tag="o")
            pieces = ((0, 5), (5, 10), (10, len(sup_spans)))
            for ti, (s0, sl) in enumerate(sup_spans):
                cols = slice(s0, s0 + sl)
                ps = psum.tile([C, SUP], f32, tag="a")
                nc.tensor.matmul(
                    out=ps[:, 0:sl], lhsT=wa_sb[:], rhs=xt_sb[:, cols],
                    start=True, stop=True,
                )
                nc.scalar.activation(
                    out=o_sb[:, cols], in_=ps[:, 0:sl],
                    func=mybir.ActivationFunctionType.Relu,
                )
                for p0, p1 in pieces:
                    if ti == p1 - 1:
                        lo = sup_spans[p0][0]
                        hi = sup_spans[p1 - 1][0] + sup_spans[p1 - 1][1]
                        nc.sync.dma_start(
                            out=osh[:, lo:hi], in_=o_sb[:, lo:hi]
                        )
    nc.compile()
    return nc


# ---------------------------------------------------------------------------
# host glue
# ---------------------------------------------------------------------------

def _numpy_fallback(x, edge_index, W, b):
    src, dst = edge_index[0], edge_index[1]
    V1 = W[:, :C] - W[:, C:]
    V2 = W[:, C:]
    A = x @ V1.T + b
    g = x @ V2.T
    out = np.full((x.shape[0], C), -np.inf, dtype=np.float32)
    msg = np.maximum(A[dst] + g[src], 0.0)
    np.maximum.at(out, dst, msg)
    return np.where(np.isneginf(out), 0.0, out).astype(np.float32)


def _run_spmd(nc, in_maps):
    # the shared axon device occasionally reports a transient
    # NRT_EXEC_UNIT_UNRECOVERABLE on a cold first launch; retry once
    import time
    from concourse.bass_utils import run_bass_kernel_spmd

    try:
        return run_bass_kernel_spmd(nc, in_maps, core_ids=list(range(N_CORES)))
    except Exception:
        time.sleep(10.0)
        return run_bass_kernel_spmd(nc, in_maps, core_ids=list(range(N_CORES)))


def kernel(x, edge_index, edge_attr, W, b):
    x = np.ascontiguousarray(x, dtype=np.float32)
    edge_index = np.ascontiguousarray(edge_index, dtype=np.int32)
    W = np.ascontiguousarray(W, dtype=np.float32)
    b = np.ascontiguousarray(b, dtype=np.float32)

    expected_dst = np.repeat(np.arange(N_NODES, dtype=np.int32), DEG)
    if (
        x.shape != (N_NODES, C)
        or edge_index.shape != (2, N_NODES * DEG)
        or not np.array_equal(edge_index[1], expected_dst)
        or edge_index[0].min() < 0
        or edge_index[0].max() >= N_NODES
    ):
        return _numpy_fallback(x, edge_index, W, b)

    src = edge_index[0].astype(np.int64)
    dst = edge_index[1].astype(np.int64)

    ek = edge_index.tobytes()
    if _cache.get("edge_key") != hash(ek):
        _cache["cfg"], _cache["host"] = _host_prep(src, dst)
        _cache["edge_key"] = hash(ek)
    cfg, host = _cache["cfg"], _cache["host"]
    if _cache.get("gather_cfg") != cfg:
        _cache["gather"] = _build_gather(cfg)
        _cache["gather_cfg"] = cfg
    if "dense" not in _cache:
        _cache["dense"] = _build_dense()

    W1, W2 = W[:, :C], W[:, C:]
    # wlo/whi: lhsT columns p -> channel p%32 (+32)
    wlo = np.ascontiguousarray(W2[np.tile(np.arange(32), 4)].T).astype(BF16)
    whi = np.ascontiguousarray(W2[np.tile(np.arange(32, 64), 4)].T).astype(BF16)

    xb = x.astype(BF16)
    in1 = []
    for c in range(N_CORES):
        xt = np.zeros((C, NLOC), dtype=BF16)
        xt[:, :NSH] = xb[c * NSH : (c + 1) * NSH].T
        in1.append(
            {"xt": xt, "wlo": wlo, "whi": whi, "idx": host["idx"][c]}
        )
    r1 = _run_spmd(_cache["gather"], in1)

    # decode per-core partials and merge (max) on host
    rtot = host["rtot"]
    rankb = host["rankb"]
    mfull = np.full((N_NODES, C), -np.inf, dtype=np.float32)
    for c in range(N_CORES):
        part = (
            r1.results[c]["osh"].reshape(128, rtot, 2).astype(np.float32)
        )
        nodes, ks = host["core_nodes"][c]
        ss = host["s_nc"][c, nodes]
        rr = rankb[ks] + host["q_nc"][c, nodes]
        vals = np.empty((len(nodes), C), dtype=np.float32)
        for s in range(NSTR):
            sel = ss == s
            if not sel.any():
                continue
            blk = part[32 * s : 32 * (s + 1), rr[sel], :]  # [32, m, 2]
            vals[sel, 0:32] = blk[:, :, 0].T
            vals[sel, 32:64] = blk[:, :, 1].T
        mfull[nodes] = np.maximum(mfull[nodes], vals)

    # ---- Launch 2: node-sharded relu(A + M) ----
    wa = np.concatenate([(W1 - W2).T, np.eye(C, dtype=np.float32)], axis=0)
    wa = wa.astype(BF16)
    mb = mfull + b  # bias folded into the identity block's rhs rows
    in2 = []
    for c in range(N_CORES):
        xt2 = np.zeros((2 * C, NLOC), dtype=BF16)
        xt2[:C, :NSH] = xb[c * NSH : (c + 1) * NSH].T
        xt2[C:, :NSH] = mb[c * NSH : (c + 1) * NSH].T.astype(BF16)
        in2.append({"xt": xt2, "wa": wa})
    r2 = _run_spmd(_cache["dense"], in2)

    out = np.empty((N_CORES, NSH, C), dtype=np.float32)
    for c in range(N_CORES):
        out[c] = r2.results[c]["osh"][:, :NSH].T.astype(np.float32)
    _cache["last_results"] = (r1, r2)
    return out.reshape(N_NODES, C)


# revision 13
# speedup vs baseline: 2.3805x; 1.0981x over previous
"""EdgeConv (PyG, aggr='max') Trainium2 kernel, 8-core SPMD.

Math: out_i = max_{e: dst(e)=i} relu(x_i @ W1.T + (x_src(e) - x_i) @ W2.T + b)
with W = [W1 | W2].  Rewriting:
    msg_e = A_i + g_src(e),  A = x @ (W1-W2).T + b,  g = x @ W2.T
Since A_i is constant within segment i and relu is monotone:
    out_i = relu(A_i + max_e g_src(e))

Sharding: edges are partitioned across cores BY SOURCE RANGE (core c owns
srcs [6250c, 6250(c+1))), so each core's entire gather table is its own
locally-computed g-shard kept in SBUF -- no HBM gather at all.  The table is
channel-paired int32 [128, 6400]: partition p holds channels (p%32, p%32+32)
packed as 2xbf16, replicated over 4 independent 32-partition "streams".  A
single GPSIMD ap_gather column then fetches a full 64-channel row for 4
different edges at once (one per stream) at ~0.35 ns/edge -- 4x cheaper than
the DMA-descriptor path and on an otherwise idle engine.

Each core's destination nodes are grouped by their per-core edge count k
(host-side reorder); the segment max becomes regular k-window max trees on
DVE over the gathered columns.  Per-core partial maxes [128, R] are written
out; the host un-permutes and np.maximum-merges them across cores (pure
unshard glue).  A second small node-sharded launch computes
relu(A + merged_max) and writes the final bf16 output.

Launch 1 (gather): xt/wlo/whi -> PE builds the paired table; 4 ap_gather
chunks of ~6400 columns pipeline with DVE max-tree spans; osh partials.
Launch 2 (dense): 13 supertile matmuls for A, DVE adds the merged max,
ACT applies relu.
"""

import math

import numpy as np
import ml_dtypes

BF16 = ml_dtypes.bfloat16

N_NODES = 50000
DEG = 16
C = 64
N_CORES = 8
NSH = N_NODES // N_CORES  # 6250 src nodes per core
NLOC = 6400  # padded local node count (table columns / L2 shard columns)
NSTR = 4  # gather streams (32 partitions each)
KMAX = DEG
SUP = 512  # supertile columns (one PSUM bank)
KC_TARGET = NLOC  # gather chunk columns ~ table size (cost floor)

_cache = {}


# ---------------------------------------------------------------------------
# host-side layout
# ---------------------------------------------------------------------------

def _host_prep(src, dst):
    """Compute the SPMD-uniform column layout from the actual edge list.

    Returns cfg (hashable, shapes for the kernel builder) and per-core host
    data (idx arrays, decode maps).
    """
    E = src.shape[0]
    core = (src // NSH).astype(np.int64)
    n64 = dst.astype(np.int64)
    kmat = np.bincount(n64 * N_CORES + core, minlength=N_NODES * N_CORES)
    kmat = kmat.reshape(N_NODES, N_CORES)  # [N, 8] per-(node, core) edge count

    # group nodes by k per core; round-robin nodes over 4 streams
    s_nc = np.zeros((N_CORES, N_NODES), np.int64)
    q_nc = np.zeros((N_CORES, N_NODES), np.int64)
    caps = np.zeros(KMAX + 1, np.int64)  # per-(k, stream) node capacity
    core_nodes = []  # per core: (nodes, k of nodes)
    for c in range(N_CORES):
        kc = kmat[:, c]
        nz = np.nonzero(kc)[0]
        core_nodes.append((nz, kc[nz]))
        for k in range(1, KMAX + 1):
            nodes = nz[kc[nz] == k]
            m = len(nodes)
            if m == 0:
                continue
            r = np.arange(m)
            s_nc[c, nodes] = r % NSTR
            q_nc[c, nodes] = r // NSTR
            caps[k] = max(caps[k], -(-m // NSTR))
    # pad caps so every k-group is 16-column aligned; chunk boundaries are
    # additionally restricted to 128-aligned node starts below (the GPSIMD
    # gather needs 4B-aligned index slices)
    for k in range(1, KMAX + 1):
        if caps[k]:
            step = 16 // math.gcd(k, 16)
            caps[k] = -(-caps[k] // step) * step
    # group layout order: k=2 last (fewest DVE ops per column, so the
    # post-last-gather reduce trail is shortest)
    order = [1] + list(range(3, KMAX + 1)) + [2]
    offk = np.zeros(KMAX + 1, np.int64)
    rankb = np.zeros(KMAX + 1, np.int64)
    off = rk = 0
    for k in order:
        offk[k] = off
        rankb[k] = rk
        off += caps[k] * k
        rk += caps[k]
    ctot = int(off)
    rtot = int(rk)

    # node-start columns in column order (for chunk boundary search)
    starts = np.concatenate(
        [offk[k] + np.arange(caps[k]) * k for k in order if caps[k]]
    )
    # chunk boundaries: near equal quarters, node-aligned and 16-aligned
    nch = max(1, int(round(ctot / KC_TARGET)))
    aligned = starts[starts % 128 == 0]
    bounds = [0]
    for i in range(1, nch):
        t = ctot * i // nch
        j = int(np.argmin(np.abs(aligned - t)))
        b = int(aligned[j])
        if b <= bounds[-1]:
            continue
        bounds.append(b)
    bounds.append(ctot)
    # chunk rank bounds
    rb = [int(np.searchsorted(starts, b)) for b in bounds]
    chunks = []
    for i in range(len(bounds) - 1):
        chunks.append((bounds[i], bounds[i + 1] - bounds[i], rb[i], rb[i + 1]))

    # reduce spans: (chunk_idx, col0, n_nodes, k, rank0), split at chunk bounds
    spans = []
    for k in order:
        if not caps[k]:
            continue
        g0, g1 = int(offk[k]), int(offk[k] + caps[k] * k)
        for ci, (b0, kc_, _, _) in enumerate(chunks):
            lo, hi = max(g0, b0), min(g1, b0 + kc_)
            if lo >= hi:
                continue
            # split long spans so the reduce->osh tail pipelines
            nn_all = (hi - lo) // k
            r0 = int(rankb[k] + (lo - g0) // k)
            p0 = 0
            while p0 < nn_all:
                nn = min(1024, nn_all - p0)
                spans.append((ci, lo + p0 * k, nn, k, r0 + p0))
                p0 += nn

    # per-edge column assignment
    key = core * N_NODES + n64
    order = np.argsort(key, kind="stable")
    sk = key[order]
    first_new = np.r_[True, sk[1:] != sk[:-1]]
    run_id = np.cumsum(first_new) - 1
    run_start = np.nonzero(first_new)[0]
    j_sorted = np.arange(E) - run_start[run_id]
    occ = np.empty(E, np.int64)
    occ[order] = j_sorted
    k_e = kmat[n64, core]
    col_e = offk[k_e] + q_nc[core, n64] * k_e + occ
    s_e = s_nc[core, n64]
    ls_e = (src - core * NSH).astype(np.int64)

    idxs = np.zeros((N_CORES, NSTR, ctot), np.int16)
    idxs[core, s_e, col_e] = ls_e

    # wrapped idx layout [128, ctot//16]: group g (partitions 16g..16g+16)
    # carries stream g//2's list, element j at [16g + j%16, j//16]
    idx_wrapped = np.empty((N_CORES, 128, ctot // 16), np.int16)
    for c in range(N_CORES):
        a = idxs[c].reshape(NSTR, ctot // 16, 16)
        for g in range(8):
            idx_wrapped[c, 16 * g : 16 * (g + 1), :] = a[g // 2].T

    cfg = (ctot, rtot, tuple(chunks), tuple(spans))
    host = {
        "idx": idx_wrapped,
        "core_nodes": core_nodes,
        "s_nc": s_nc,
        "q_nc": q_nc,
        "rankb": rankb,
        "rtot": rtot,
    }
    return cfg, host


# ---------------------------------------------------------------------------
# launch 1: src-sharded gather + per-core segment max partials
# ---------------------------------------------------------------------------

def _build_gather(cfg):
    import concourse.bacc as bacc
    import concourse.mybir as mybir
    from concourse.tile import TileContext

    ctot, rtot, chunks, spans = cfg
    kc_max = max(kc for _, kc, _, _ in chunks)

    nc = bacc.Bacc("TRN2", target_bir_lowering=False, debug=False)
    f32 = mybir.dt.float32
    bf16 = mybir.dt.bfloat16
    i32 = mybir.dt.int32
    i16 = mybir.dt.int16
    mx = mybir.AluOpType.max

    xt = nc.dram_tensor("xt", [C, NLOC], bf16, kind="ExternalInput")
    wlo = nc.dram_tensor("wlo", [C, 128], bf16, kind="ExternalInput")
    whi = nc.dram_tensor("whi", [C, 128], bf16, kind="ExternalInput")
    idx = nc.dram_tensor("idx", [128, ctot // 16], i16, kind="ExternalInput")
    osh = nc.dram_tensor("osh", [128, 2 * rtot], bf16, kind="ExternalOutput")

    nsup = NLOC // SUP  # 12.5 -> handle tail below
    sup_spans = [(i * SUP, SUP) for i in range(NLOC // SUP)]
    if NLOC % SUP:
        sup_spans.append((NLOC // SUP * SUP, NLOC % SUP))

    with TileContext(nc) as tc:
        with (
            tc.tile_pool(name="const", bufs=1) as cpool,
            tc.tile_pool(name="sbuf", bufs=1) as pool,
            tc.tile_pool(name="gat", bufs=3) as gpool,
            tc.tile_pool(name="psum", bufs=4, space="PSUM") as psum,
        ):
            # xt's first piece leads the DMA queue so PE starts ASAP; the
            # gather index list only matters once the table is built
            xt_sb = pool.tile([C, NLOC], bf16, tag="xt")
            nc.sync.dma_start(out=xt_sb[:, 0:SUP], in_=xt[:, 0:SUP])
            wlo_sb = cpool.tile([C, 128], bf16)
            nc.sync.dma_start(out=wlo_sb[:], in_=wlo[:])
            whi_sb = cpool.tile([C, 128], bf16)
            nc.sync.dma_start(out=whi_sb[:], in_=whi[:])
            for a, b in ((SUP, 3 * SUP), (3 * SUP, 8 * SUP), (8 * SUP, NLOC)):
                nc.sync.dma_start(out=xt_sb[:, a:b], in_=xt[:, a:b])
            idx_sb = pool.tile([128, ctot // 16], i16, tag="idx")
            nc.sync.dma_start(out=idx_sb[:], in_=idx[:])

            # paired g table: int32[p, n] = (g[n, p%32] , g[n, p%32+32]);
            # lo/hi matmuls land in a 2-bank PSUM tile, one interleaving
            # eviction per tile, alternating ACT/DVE
            tbl = pool.tile([128, NLOC], i32, tag="tbl")
            tbl_bf = tbl[:].bitcast(bf16).rearrange("p (n t) -> p n t", t=2)
            for ti, (s0, sl) in enumerate(sup_spans):
                cols = slice(s0, s0 + sl)
                ps = psum.tile([128, 2, SUP], f32, tag="p2")
                nc.tensor.matmul(
                    out=ps[:, 0, 0:sl], lhsT=wlo_sb[:], rhs=xt_sb[:, cols],
                    start=True, stop=True,
                )
                nc.tensor.matmul(
                    out=ps[:, 1, 0:sl], lhsT=whi_sb[:], rhs=xt_sb[:, cols],
                    start=True, stop=True,
                )
                src_ap = ps[:, :, 0:sl].rearrange("p h n -> p n h")
                if ti % 2 == 0:
                    nc.scalar.copy(out=tbl_bf[:, cols, :], in_=src_ap)
                else:
                    nc.vector.tensor_copy(out=tbl_bf[:, cols, :], in_=src_ap)

            osh_buf = pool.tile([128, rtot], i32, tag="oshb")
            osh_bf = osh_buf[:].bitcast(bf16).rearrange("p (n t) -> p n t", t=2)

            for ci, (b0, kc, cr0, cr1) in enumerate(chunks):
                g = gpool.tile([128, kc_max], i32, tag="g")
                nc.gpsimd.ap_gather(
                    out_ap=g[:, 0:kc].rearrange("p (n d) -> p n d", d=1),
                    in_ap=tbl[:].rearrange("p (n d) -> p n d", d=1),
                    idxs_ap=idx_sb[:, b0 // 16 : (b0 + kc) // 16],
                    channels=128, num_elems=NLOC, d=1, num_idxs=kc,
                )
                g_bf = g[:].bitcast(bf16)
                for sci, col0, nn, k, r0 in spans:
                    if sci != ci:
                        continue
                    l0 = col0 - b0
                    v = g_bf[:, 2 * l0 : 2 * (l0 + nn * k)].rearrange(
                        "p (n k t) -> p n k t", k=k, t=2
                    )
                    dst = osh_bf[:, r0 : r0 + nn, :]
                    if k == 1:
                        nc.vector.tensor_copy(out=dst, in_=v[:, :, 0, :])
                        continue
                    j = k
                    while j > 2:
                        if j % 2:
                            nc.vector.tensor_tensor(
                                out=v[:, :, 0, :], in0=v[:, :, 0, :],
                                in1=v[:, :, j - 1, :], op=mx,
                            )
                            j -= 1
                        m = j // 2
                        if j > 2:
                            nc.vector.tensor_tensor(
                                out=v[:, :, 0:m, :], in0=v[:, :, 0:m, :],
                                in1=v[:, :, m : 2 * m, :], op=mx,
                            )
                            j = m
                    nc.vector.tensor_tensor(
                        out=dst, in0=v[:, :, 0, :], in1=v[:, :, 1, :], op=mx
                    )
                for sci, col0, nn, k, r0 in spans:
                    if sci != ci:
                        continue
                    nc.sync.dma_start(
                        out=osh[:, 2 * r0 : 2 * (r0 + nn)],
                        in_=osh_buf[:].bitcast(bf16)[:, 2 * r0 : 2 * (r0 + nn)],
                    )
    nc.compile()
    return nc


# ---------------------------------------------------------------------------
# launch 2: node-sharded A + merged max, relu
# ---------------------------------------------------------------------------

def _build_dense():
    """relu(A + M) node-sharded.  The merged max M (with bias pre-added) is
    carried as extra rhs rows against an identity block in lhsT, so a single
    matmul yields A + M + b in PSUM and ACT applies the relu directly."""
    import concourse.bacc as bacc
    import concourse.mybir as mybir
    from concourse.tile import TileContext

    nc = bacc.Bacc("TRN2", target_bir_lowering=False, debug=False)
    f32 = mybir.dt.float32
    bf16 = mybir.dt.bfloat16

    # rows 0..63 = x-shard^T, rows 64..127 = (M + b)^T
    xt = nc.dram_tensor("xt", [2 * C, NLOC], bf16, kind="ExternalInput")
    wa = nc.dram_tensor("wa", [2 * C, C], bf16, kind="ExternalInput")
    osh = nc.dram_tensor("osh", [C, NLOC], bf16, kind="ExternalOutput")

    sup_spans = [(i * SUP, SUP) for i in range(NLOC // SUP)]
    if NLOC % SUP:
        sup_spans.append((NLOC // SUP * SUP, NLOC % SUP))

    with TileContext(nc) as tc:
        with (
            tc.tile_pool(name="const", bufs=1) as cpool,
            tc.tile_pool(name="sbuf", bufs=1) as pool,
            tc.tile_pool(name="psum", bufs=4, space="PSUM") as psum,
        ):
            wa_sb = cpool.tile([2 * C, C], bf16)
            nc.sync.dma_start(out=wa_sb[:], in_=wa[:])
            # preload the ACT function table while inputs stream in
            warm = cpool.tile([1, 2], f32)
            nc.vector.memset(warm[:], 0.0)
            warm2 = cpool.tile([1, 2], f32)
            nc.scalar.activation(
                out=warm2[:], in_=warm[:],
                func=mybir.ActivationFunctionType.Relu,
            )
            xt_sb = pool.tile([2 * C, NLOC], bf16, tag="xt")
            for a, b in ((0, SUP), (SUP, 4 * SUP), (4 * SUP, 8 * SUP),
                         (8 * SUP, NLOC)):
                nc.sync.dma_start(out=xt_sb[:, a:b], in_=xt[:, a:b])
            o_sb = pool.tile([C, NLOC], bf16, tag="o")
            pieces = ((0, 5), (5, 10), (10, len(sup_spans)))
            for ti, (s0, sl) in enumerate(sup_spans):
                cols = slice(s0, s0 + sl)
                ps = psum.tile([C, SUP], f32, tag="a")
                nc.tensor.matmul(
                    out=ps[:, 0:sl], lhsT=wa_sb[:], rhs=xt_sb[:, cols],
                    start=True, stop=True,
                )
                if ti % 2 == 0:
                    nc.scalar.activation(
                        out=o_sb[:, cols], in_=ps[:, 0:sl],
                        func=mybir.ActivationFunctionType.Relu,
                    )
                else:
                    nc.vector.tensor_relu(out=o_sb[:, cols], in_=ps[:, 0:sl])
                for p0, p1 in pieces:
                    if ti == p1 - 1:
                        lo = sup_spans[p0][0]
                        hi = sup_spans[p1 - 1][0] + sup_spans[p1 - 1][1]
                        nc.sync.dma_start(
                            out=osh[:, lo:hi], in_=o_sb[:, lo:hi]
                        )
    nc.compile()
    return nc


# ---------------------------------------------------------------------------
# host glue
# ---------------------------------------------------------------------------

def _numpy_fallback(x, edge_index, W, b):
    src, dst = edge_index[0], edge_index[1]
    V1 = W[:, :C] - W[:, C:]
    V2 = W[:, C:]
    A = x @ V1.T + b
    g = x @ V2.T
    out = np.full((x.shape[0], C), -np.inf, dtype=np.float32)
    msg = np.maximum(A[dst] + g[src], 0.0)
    np.maximum.at(out, dst, msg)
    return np.where(np.isneginf(out), 0.0, out).astype(np.float32)


def _run_spmd(nc, in_maps):
    # the shared axon device occasionally reports a transient
    # NRT_EXEC_UNIT_UNRECOVERABLE on a cold first launch; retry once
    import time
    from concourse.bass_utils import run_bass_kernel_spmd

    try:
        return run_bass_kernel_spmd(nc, in_maps, core_ids=list(range(N_CORES)))
    except Exception:
        time.sleep(10.0)
        return run_bass_kernel_spmd(nc, in_maps, core_ids=list(range(N_CORES)))


def kernel(x, edge_index, edge_attr, W, b):
    x = np.ascontiguousarray(x, dtype=np.float32)
    edge_index = np.ascontiguousarray(edge_index, dtype=np.int32)
    W = np.ascontiguousarray(W, dtype=np.float32)
    b = np.ascontiguousarray(b, dtype=np.float32)

    expected_dst = np.repeat(np.arange(N_NODES, dtype=np.int32), DEG)
    if (
        x.shape != (N_NODES, C)
        or edge_index.shape != (2, N_NODES * DEG)
        or not np.array_equal(edge_index[1], expected_dst)
        or edge_index[0].min() < 0
        or edge_index[0].max() >= N_NODES
    ):
        return _numpy_fallback(x, edge_index, W, b)

    src = edge_index[0].astype(np.int64)
    dst = edge_index[1].astype(np.int64)

    ek = edge_index.tobytes()
    if _cache.get("edge_key") != hash(ek):
        _cache["cfg"], _cache["host"] = _host_prep(src, dst)
        _cache["edge_key"] = hash(ek)
    cfg, host = _cache["cfg"], _cache["host"]
    if _cache.get("gather_cfg") != cfg:
        _cache["gather"] = _build_gather(cfg)
        _cache["gather_cfg"] = cfg
    if "dense" not in _cache:
        _cache["dense"] = _build_dense()

    W1, W2 = W[:, :C], W[:, C:]
    # wlo/whi: lhsT columns p -> channel p%32 (+32)
    wlo = np.ascontiguousarray(W2[np.tile(np.arange(32), 4)].T).astype(BF16)
    whi = np.ascontiguousarray(W2[np.tile(np.arange(32, 64), 4)].T).astype(BF16)

    xb = x.astype(BF16)
    in1 = []
    for c in range(N_CORES):
        xt = np.zeros((C, NLOC), dtype=BF16)
        xt[:, :NSH] = xb[c * NSH : (c + 1) * NSH].T
        in1.append(
            {"xt": xt, "wlo": wlo, "whi": whi, "idx": host["idx"][c]}
        )
    r1 = _run_spmd(_cache["gather"], in1)

    # decode per-core partials and merge (max) on host
    rtot = host["rtot"]
    rankb = host["rankb"]
    mfull = np.full((N_NODES, C), -np.inf, dtype=np.float32)
    for c in range(N_CORES):
        part = (
            r1.results[c]["osh"].reshape(128, rtot, 2).astype(np.float32)
        )
        nodes, ks = host["core_nodes"][c]
        ss = host["s_nc"][c, nodes]
        rr = rankb[ks] + host["q_nc"][c, nodes]
        vals = np.empty((len(nodes), C), dtype=np.float32)
        for s in range(NSTR):
            sel = ss == s
            if not sel.any():
                continue
            blk = part[32 * s : 32 * (s + 1), rr[sel], :]  # [32, m, 2]
            vals[sel, 0:32] = blk[:, :, 0].T
            vals[sel, 32:64] = blk[:, :, 1].T
        mfull[nodes] = np.maximum(mfull[nodes], vals)

    # ---- Launch 2: node-sharded relu(A + M) ----
    wa = np.concatenate([(W1 - W2).T, np.eye(C, dtype=np.float32)], axis=0)
    wa = wa.astype(BF16)
    mb = mfull + b  # bias folded into the identity block's rhs rows
    in2 = []
    for c in range(N_CORES):
        xt2 = np.zeros((2 * C, NLOC), dtype=BF16)
        xt2[:C, :NSH] = xb[c * NSH : (c + 1) * NSH].T
        xt2[C:, :NSH] = mb[c * NSH : (c + 1) * NSH].T.astype(BF16)
        in2.append({"xt": xt2, "wa": wa})
    r2 = _run_spmd(_cache["dense"], in2)

    out = np.empty((N_CORES, NSH, C), dtype=np.float32)
    for c in range(N_CORES):
        out[c] = r2.results[c]["osh"][:, :NSH].T.astype(np.float32)
    _cache["last_results"] = (r1, r2)
    return out.reshape(N_NODES, C)
